# revision 1
# baseline (speedup 1.0000x reference)
"""Distributed 8-layer dense transformer on 8 TRN2 NeuronCores.

Sharding: context-parallel. Each core owns 256 contiguous tokens (4 chunks
per batch element x 2 batch elements = 8 cores). All weights replicated.
Per layer, each 4-core batch group AllGathers K^T then V (fp16, ~0.5MB each,
pipelined); everything else is local. The final vocab projection is computed
per-core for its own 256 tokens.

Layouts: activations are feature-major (x^T: [D, T], partition = feature).
V is produced token-major via "reversed" matmuls (activations stationary,
weights moving) and carries an appended ones-column per head so the softmax
denominator falls out of the attention matmul for free. Causality uses
per-core multiplicative 0/1 masks (inputs), keeping one SPMD instruction
stream across all cores.

Precision: fp16 weights/activations, bf16 exp tiles and V, f32 residual
stream / LN stats / PSUM accumulation.

PSUM rule learned the hard way: a matmul with start=True clears has_written
for its whole PSUM bank, so two multi-step accumulation groups must never
share a bank while interleaved.
"""

import numpy as np
import ml_dtypes

import concourse.bass as bass
import concourse.mybir as mybir
import concourse.tile as tile
import concourse.bacc as bacc
from concourse.bass_utils import run_bass_kernel_spmd

F32 = mybir.dt.float32
F16 = mybir.dt.float16
BF16 = mybir.dt.bfloat16
I32 = mybir.dt.int32
AF = mybir.ActivationFunctionType
ALU = mybir.AluOpType

L, D, H, DK, F, V, S, B = 8, 1024, 16, 64, 4096, 32000, 1024, 2
NCORES = 8
G = 4
T = (B * S) // NCORES   # 256
NT = T // 128           # 2
ND = D // 128           # 8
NF = F // 128           # 32
NSLOT = (G * T) // 128  # 8
VO = DK + 1             # 65
EPS = 1e-5
SCALE = 1.0 / np.sqrt(DK)

KV_K = 1024 * T          # K^T elements [1024, 256]
KV_V = T * (H * VO)      # V elements [256, 1040]

PC_BQ, PC_BK, PC_BO, PC_B1, PC_B2 = 0, 8, 16, 24, 56
PC_G1, PC_BE1, PC_G2, PC_BE2 = 64, 72, 80, 88
NPC = 96

_cache = {}
DEBUG = False


def build():
    nc = bacc.Bacc("TRN2", target_bir_lowering=False, debug=False,
                   num_devices=NCORES)
    if DEBUG:
        dbgx_e = nc.dram_tensor("dbgx", [9, 128, ND, T], F32,
                                kind="ExternalOutput")
        dbgh_e = nc.dram_tensor("dbgh", [4, 128, ND, T], F16,
                                kind="ExternalOutput")
        dbge_e = nc.dram_tensor("dbge", [H, 128, NSLOT, T], BF16,
                                kind="ExternalOutput")

    ids_e = nc.dram_tensor("ids", [128, NT], I32, kind="ExternalInput")
    tok_e = nc.dram_tensor("tok_emb", [V, D], F32, kind="ExternalInput")
    pos_e = nc.dram_tensor("pos_t", [128, ND, T], F32, kind="ExternalInput")
    mask_e = nc.dram_tensor("masks", [128, NSLOT, T], BF16, kind="ExternalInput")
    wq_e = nc.dram_tensor("Wq", [L, D, D], F16, kind="ExternalInput")
    wk_e = nc.dram_tensor("Wk", [L, D, D], F16, kind="ExternalInput")
    wv_e = nc.dram_tensor("Wv", [L, D, D], F16, kind="ExternalInput")
    wo_e = nc.dram_tensor("Wo", [L, D, D], F16, kind="ExternalInput")
    w1_e = nc.dram_tensor("W1", [L, D, F], F16, kind="ExternalInput")
    w2_e = nc.dram_tensor("W2", [L, F, D], F16, kind="ExternalInput")
    wout_e = nc.dram_tensor("Wout", [D, V], F16, kind="ExternalInput")
    par_e = nc.dram_tensor("par", [L, 128, NPC], F32, kind="ExternalInput")
    bv_e = nc.dram_tensor("bv", [L, 1, D], F32, kind="ExternalInput")
    fin_e = nc.dram_tensor("fin", [128, 16], F32, kind="ExternalInput")
    bout_e = nc.dram_tensor("bout", [1, V], F32, kind="ExternalInput")
    out_e = nc.dram_tensor("out", [T, V], F32, kind="ExternalOutput")

    ident_c = nc.inline_tensor(np.eye(128, dtype=np.float32), name="identc")
    ones_c = nc.inline_tensor(np.ones((128, 128), dtype=np.float32), name="onesc")

    with tile.TileContext(nc) as tc:
        with (
            tc.tile_pool(name="persist", bufs=1) as pp,
            tc.tile_pool(name="wp", bufs=4) as wp,
            tc.tile_pool(name="w2p", bufs=2) as w2p,
            tc.tile_pool(name="ep", bufs=3) as ep,
            tc.tile_pool(name="small", bufs=3) as sp,
            tc.tile_pool(name="tmpp", bufs=4) as tp,
            tc.tile_pool(name="outp", bufs=4) as op_,
            tc.tile_pool(name="embp", bufs=1) as embp,
            tc.tile_pool(name="ps_m", bufs=4, space="PSUM") as ps_m,
            tc.tile_pool(name="ps_o", bufs=2, space="PSUM") as ps_o,
            tc.tile_pool(name="ps_u", bufs=2, space="PSUM") as ps_u,
            tc.tile_pool(name="dram", bufs=1, space="DRAM") as dp,
        ):
            x_sb = pp.tile([128, ND, T], F32, name="x_sb")
            h_sb = pp.tile([128, ND, T], F16, name="h_sb")
            q_sb = pp.tile([128, ND, T], F16, name="q_sb")
            o_sb = pp.tile([128, ND, T], F16, name="o_sb")
            ktl_sb = pp.tile([128, ND, T], F16, name="ktl_sb")
            vl_sb = pp.tile([128, NT, H * VO], BF16, name="vl_sb")
            kt_sb = pp.tile([128, ND, G * T], F16, name="kt_sb")
            v_sb = pp.tile([128, NSLOT, H * VO], BF16, name="v_sb")
            r_sb = pp.tile([128, NF, T], F16, name="r_sb")
            mask_sb = pp.tile([128, NSLOT, T], BF16, name="mask_sb")
            pos_sb = pp.tile([128, ND, T], F32, name="pos_sb")
            ids_sb = pp.tile([128, NT], I32, name="ids_sb")
            id_sb = pp.tile([128, 128], F32, name="id_sb")
            ones_sb = pp.tile([128, 128], F32, name="ones_sb")
            fin_sb = pp.tile([128, 16], F32, name="fin_sb")
            bvbc_sb = pp.tile([128, D], F32, name="bvbc_sb")
            eps_sb = pp.tile([1, 1], F32, name="eps_sb")

            k_local = dp.tile([KV_K], F16, name="k_local")
            v_local = dp.tile([KV_V], F16, name="v_local")
            k_gath = dp.tile([G, KV_K], F16, name="k_gath")
            v_gath = dp.tile([G, KV_V], F16, name="v_gath")

            nc.sync.dma_start(out=ids_sb[:], in_=ids_e[:])
            nc.sync.dma_start(out=id_sb[:], in_=ident_c[:])
            nc.sync.dma_start(out=ones_sb[:], in_=ones_c[:])
            nc.sync.dma_start(out=pos_sb[:], in_=pos_e[:])
            nc.sync.dma_start(out=mask_sb[:], in_=mask_e[:])
            nc.sync.dma_start(out=fin_sb[:], in_=fin_e[:])
            nc.vector.memset(vl_sb[:], 1.0)
            nc.vector.memset(eps_sb[:], EPS)

            # ---- embedding: gather + transpose to feature-major + pos add
            for tb in range(NT):
                emb = embp.tile([128, D], F32, name="emb")
                nc.gpsimd.indirect_dma_start(
                    out=emb[:], out_offset=None, in_=tok_e[:],
                    in_offset=bass.IndirectOffsetOnAxis(
                        ap=ids_sb[:, tb:tb + 1], axis=0))
                for dt in range(ND):
                    tps = ps_u.tile([128, 512], F32, name="tps", tag="psu")
                    nc.tensor.transpose(
                        tps[:, 0:128], emb[:, 128 * dt:128 * dt + 128], id_sb[:])
                    nc.vector.tensor_add(
                        x_sb[:, dt, 128 * tb:128 * tb + 128],
                        tps[:, 0:128],
                        pos_sb[:, dt, 128 * tb:128 * tb + 128])
            if DEBUG:
                nc.sync.dma_start(out=dbgx_e[0], in_=x_sb[:])

            def layernorm(par_ap, gcol, bcol, out_sb):
                """x_sb (f32) -> out_sb (f16). Sum and sumsq accumulation
                groups live in different PSUM banks (start=True clears the
                whole bank's has_written)."""
                st1 = ps_u.tile([1, 512], F32, name="st1", tag="psu")
                st2 = ps_u.tile([1, 512], F32, name="st2", tag="psu")
                for k in range(ND):
                    nc.tensor.matmul(st1[0:1, 0:T], ones_sb[:, 0:1],
                                     x_sb[:, k, :], start=(k == 0),
                                     stop=(k == ND - 1))
                for k in range(ND):
                    sq = tp.tile([128, T], F32, name="sq", tag="lntmp")
                    nc.scalar.activation(sq[:], x_sb[:, k, :], AF.Square)
                    nc.tensor.matmul(st2[0:1, 0:T], ones_sb[:, 0:1],
                                     sq[:], start=(k == 0), stop=(k == ND - 1))
                mr = sp.tile([1, 512], F32, name="mr", tag="mr")
                t1 = sp.tile([1, T], F32, name="lns1", tag="lns")
                t2 = sp.tile([1, T], F32, name="lns2", tag="lns")
                nc.scalar.activation(mr[0:1, 0:T], st1[0:1, 0:T], AF.Copy,
                                     scale=1.0 / D)
                nc.scalar.activation(t1[0:1, :], st2[0:1, 0:T], AF.Copy,
                                     scale=1.0 / D)
                nc.vector.tensor_mul(t2[0:1, :], mr[0:1, 0:T], mr[0:1, 0:T])
                nc.vector.tensor_sub(t1[0:1, :], t1[0:1, :], t2[0:1, :])
                nc.scalar.activation(t2[0:1, :], t1[0:1, :], AF.Sqrt,
                                     bias=eps_sb[0:1, 0:1])
                nc.vector.reciprocal(mr[0:1, T:2 * T], t2[0:1, :])
                bc = ps_u.tile([128, 512], F32, name="lnbc", tag="psu")
                nc.tensor.matmul(bc[:, 0:512], ones_sb[0:1, 0:128],
                                 mr[0:1, 0:512], start=True, stop=True)
                for k in range(ND):
                    u1 = tp.tile([128, T], F32, name="u1", tag="lntmp")
                    u2 = tp.tile([128, T], F32, name="u2", tag="lntmp")
                    nc.vector.tensor_sub(u1[:], x_sb[:, k, :], bc[:, 0:T])
                    nc.vector.tensor_mul(u2[:], u1[:], bc[:, T:2 * T])
                    nc.vector.tensor_scalar(
                        out=out_sb[:, k, :], in0=u2[:],
                        scalar1=par_ap[:, gcol + k:gcol + k + 1],
                        scalar2=par_ap[:, bcol + k:bcol + k + 1],
                        op0=ALU.mult, op1=ALU.add)

            def std_proj(w_ext, l, dst_sb, bias_par, bias_col):
                """dst[:, m, :] (f16) = (h^T W)[:, m] + bias, feature-major."""
                for c in range(2):
                    slab = wp.tile([128, ND, 512], F16, name="wslab", tag="wslab")
                    nc.sync.dma_start(
                        out=slab[:],
                        in_=w_ext[l, :, 512 * c:512 * c + 512].rearrange(
                            "(k p) n -> p k n", p=128))
                    for mm in range(4):
                        m = 4 * c + mm
                        ps = ps_m.tile([128, 512], F32, name="pp", tag="psm")
                        for k in range(ND):
                            nc.tensor.matmul(
                                ps[:, 0:T],
                                slab[:, k, 128 * mm:128 * mm + 128],
                                h_sb[:, k, :],
                                start=(k == 0), stop=(k == ND - 1))
                        nc.scalar.activation(
                            dst_sb[:, m, :], ps[:, 0:T], AF.Identity,
                            bias=bias_par[:, bias_col + m:bias_col + m + 1])

            # =================== layers ===================
            for l in range(L):
                par = sp.tile([128, NPC], F32, name="par", tag="par")
                nc.sync.dma_start(out=par[:], in_=par_e[l])
                bv_t = sp.tile([1, D], F32, name="bv_t", tag="bv")
                nc.sync.dma_start(out=bv_t[:], in_=bv_e[l])
                for c in range(2):
                    bcv = ps_u.tile([128, 512], F32, name="bcv", tag="psu")
                    nc.tensor.matmul(bcv[:], ones_sb[0:1, 0:128],
                                     bv_t[0:1, 512 * c:512 * c + 512],
                                     start=True, stop=True)
                    nc.scalar.copy(bvbc_sb[:, 512 * c:512 * c + 512], bcv[:])

                # ---- LN1
                layernorm(par, PC_G1, PC_BE1, h_sb)
                if DEBUG and l == 0:
                    nc.sync.dma_start(out=dbgh_e[0], in_=h_sb[:])

                # ---- K projection first, then its AllGather right away
                std_proj(wk_e, l, ktl_sb, par, PC_BK)
                if DEBUG and l == 0:
                    nc.sync.dma_start(out=dbgh_e[2], in_=ktl_sb[:])
                nc.sync.dma_start(
                    out=k_local[:].rearrange("(k p t) -> p k t", p=128, t=T),
                    in_=ktl_sb[:])
                nc.gpsimd.collective_compute(
                    "AllGather", ALU.bypass,
                    replica_groups=[[0, 1, 2, 3], [4, 5, 6, 7]],
                    ins=[k_local[:].opt()], outs=[k_gath[:].opt()])

                # ---- V projection (token-major, reversed) overlaps K-AG
                for c in range(2):
                    slab = wp.tile([128, ND, 512], F16, name="wslab", tag="wslab")
                    nc.sync.dma_start(
                        out=slab[:],
                        in_=wv_e[l, :, 512 * c:512 * c + 512].rearrange(
                            "(k p) n -> p k n", p=128))
                    for tb in range(NT):
                        ps = ps_m.tile([128, 512], F32, name="pp", tag="psm")
                        for k in range(ND):
                            nc.tensor.matmul(
                                ps[:], h_sb[:, k, 128 * tb:128 * tb + 128],
                                slab[:, k, :],
                                start=(k == 0), stop=(k == ND - 1))
                        dst = vl_sb[:, tb,
                                    VO * 8 * c:VO * 8 * c + VO * 8].rearrange(
                            "p (j v) -> p j v", v=VO)[:, :, 0:DK]
                        nc.vector.tensor_add(
                            dst,
                            ps[:].rearrange("p (j v) -> p j v", v=DK),
                            bvbc_sb[:, 512 * c:512 * c + 512].rearrange(
                                "p (j v) -> p j v", v=DK))
                nc.sync.dma_start(
                    out=v_local[:].rearrange("(tb p c) -> p tb c", p=128,
                                             c=H * VO),
                    in_=vl_sb[:].bitcast(F16))
                nc.gpsimd.collective_compute(
                    "AllGather", ALU.bypass,
                    replica_groups=[[0, 1, 2, 3], [4, 5, 6, 7]],
                    ins=[v_local[:].opt()], outs=[v_gath[:].opt()])

                # ---- Q projection (overlaps the AllGathers)
                std_proj(wq_e, l, q_sb, par, PC_BQ)
                if DEBUG and l == 0:
                    nc.sync.dma_start(out=dbgh_e[1], in_=q_sb[:])

                # ---- HAM-warming filler: keep TensorE busy through the
                #      AllGather stall so it stays at 2.4 GHz (K=8/8). The
                #      results are never read; each start=True overwrites.
                warm = ps_m.tile([128, 512], F32, name="warm", tag="psm")
                for _ in range(56):
                    nc.tensor.matmul(warm[:, 0:T], h_sb[:, 0, 0:128],
                                     h_sb[:, 0, :], start=True, stop=True)

                # ---- pull gathered K^T / V into SBUF
                for c in range(G):
                    nc.sync.dma_start(
                        out=kt_sb[:, :, T * c:T * c + T],
                        in_=k_gath[c].rearrange("(k p t) -> p k t", p=128, t=T))
                for c in range(G):
                    nc.sync.dma_start(
                        out=v_sb[:, 2 * c:2 * c + 2, :],
                        in_=v_gath[c].rearrange(
                            "(tb p cc) -> p tb cc", p=128,
                            cc=H * VO).bitcast(BF16))

                # ---- attention
                for h in range(H):
                    po = 64 * (h % 2)
                    pt = h // 2
                    e_t = ep.tile([128, NSLOT, T], BF16, name="e_t", tag="et")
                    for sp_ in range(NSLOT // 2):
                        sa = ps_m.tile([128, 512], F32, name="sa", tag="psm")
                        for half in range(2):
                            s = 2 * sp_ + half
                            nc.tensor.matmul(
                                sa[:, 256 * half:256 * half + 256],
                                kt_sb[po:po + 64, pt, 128 * s:128 * s + 128],
                                q_sb[po:po + 64, pt, :],
                                start=True, stop=True)
                        nc.scalar.activation(
                            e_t[:, 2 * sp_:2 * sp_ + 2, :], sa[:], AF.Exp,
                            scale=float(SCALE))
                        nc.vector.tensor_mul(
                            e_t[:, 2 * sp_:2 * sp_ + 2, :],
                            e_t[:, 2 * sp_:2 * sp_ + 2, :],
                            mask_sb[:, 2 * sp_:2 * sp_ + 2, :])
                    oo = ps_o.tile([VO, T], F32, name="oo", tag="pso")
                    for s in range(NSLOT):
                        nc.tensor.matmul(
                            oo[:], v_sb[:, s, VO * h:VO * h + VO],
                            e_t[:, s, :],
                            start=(s == 0), stop=(s == NSLOT - 1))
                    rec = sp.tile([1, T], F32, name="rec", tag="rec")
                    nc.vector.reciprocal(rec[0:1, :], oo[DK:VO, :])
                    rbc = ps_u.tile([128, 512], F32, name="rbc", tag="psu")
                    nc.tensor.matmul(rbc[0:64, 0:T], ones_sb[0:1, 0:64],
                                     rec[0:1, :], start=True, stop=True)
                    rbs = tp.tile([64, T], F32, name="rbs", tag="rbs")
                    nc.scalar.copy(rbs[:], rbc[0:64, 0:T])
                    nc.vector.tensor_mul(o_sb[po:po + 64, pt, :],
                                         oo[0:DK, :], rbs[:])
                    if DEBUG and l == 0:
                        nc.sync.dma_start(out=dbge_e[h], in_=e_t[:])

                # ---- attention output projection + residual
                for c in range(2):
                    slab = wp.tile([128, ND, 512], F16, name="wslab", tag="wslab")
                    nc.sync.dma_start(
                        out=slab[:],
                        in_=wo_e[l, :, 512 * c:512 * c + 512].rearrange(
                            "(k p) n -> p k n", p=128))
                    for mm in range(4):
                        m = 4 * c + mm
                        ps = ps_m.tile([128, 512], F32, name="pp", tag="psm")
                        for k in range(ND):
                            nc.tensor.matmul(
                                ps[:, 0:T],
                                slab[:, k, 128 * mm:128 * mm + 128],
                                o_sb[:, k, :],
                                start=(k == 0), stop=(k == ND - 1))
                        rt = tp.tile([128, T], F32, name="rt", tag="lntmp")
                        nc.scalar.activation(
                            rt[:], ps[:, 0:T], AF.Identity,
                            bias=par[:, PC_BO + m:PC_BO + m + 1])
                        nc.vector.tensor_add(x_sb[:, m, :], x_sb[:, m, :], rt[:])

                # ---- LN2
                layernorm(par, PC_G2, PC_BE2, h_sb)

                # ---- FFN W1 + relu
                for c in range(8):
                    slab = wp.tile([128, ND, 512], F16, name="wslab", tag="wslab")
                    nc.sync.dma_start(
                        out=slab[:],
                        in_=w1_e[l, :, 512 * c:512 * c + 512].rearrange(
                            "(k p) n -> p k n", p=128))
                    for mm in range(4):
                        ot = 4 * c + mm
                        ps = ps_m.tile([128, 512], F32, name="pp", tag="psm")
                        for k in range(ND):
                            nc.tensor.matmul(
                                ps[:, 0:T],
                                slab[:, k, 128 * mm:128 * mm + 128],
                                h_sb[:, k, :],
                                start=(k == 0), stop=(k == ND - 1))
                        nc.scalar.activation(
                            r_sb[:, ot, :], ps[:, 0:T], AF.Relu,
                            bias=par[:, PC_B1 + ot:PC_B1 + ot + 1])

                # ---- FFN W2 + residual
                for m in range(ND):
                    slab2 = w2p.tile([128, NF, 128], F16, name="w2slab",
                                     tag="w2slab")
                    nc.sync.dma_start(
                        out=slab2[:],
                        in_=w2_e[l, :, 128 * m:128 * m + 128].rearrange(
                            "(k p) n -> p k n", p=128))
                    ps = ps_m.tile([128, 512], F32, name="pp", tag="psm")
                    for k in range(NF):
                        nc.tensor.matmul(
                            ps[:, 0:T], slab2[:, k, :], r_sb[:, k, :],
                            start=(k == 0), stop=(k == NF - 1))
                    rt = tp.tile([128, T], F32, name="rt2", tag="lntmp")
                    nc.scalar.activation(
                        rt[:], ps[:, 0:T], AF.Identity,
                        bias=par[:, PC_B2 + m:PC_B2 + m + 1])
                    nc.vector.tensor_add(x_sb[:, m, :], x_sb[:, m, :], rt[:])
                if DEBUG:
                    nc.sync.dma_start(out=dbgx_e[1 + l], in_=x_sb[:])
                    if l == 0:
                        nc.sync.dma_start(out=dbgh_e[3], in_=o_sb[:])

            # =================== final LN + vocab projection ===================
            layernorm(fin_sb, 0, 8, h_sb)

            NVS = (V + 511) // 512
            for vs in range(NVS):
                n = min(512, V - 512 * vs)
                slab = wp.tile([128, ND, 512], F16, name="wslab", tag="wslab")
                nc.sync.dma_start(
                    out=slab[:, :, 0:n],
                    in_=wout_e[:, 512 * vs:512 * vs + n].rearrange(
                        "(k p) n -> p k n", p=128))
                bo_t = sp.tile([1, 512], F32, name="bo_t", tag="bo")
                nc.sync.dma_start(out=bo_t[0:1, 0:n],
                                  in_=bout_e[0:1, 512 * vs:512 * vs + n])
                bb = ps_u.tile([128, 512], F32, name="bb", tag="psu")
                nc.tensor.matmul(bb[:, 0:n], ones_sb[0:1, 0:128],
                                 bo_t[0:1, 0:n], start=True, stop=True)
                bbs = op_.tile([128, 512], F32, name="bbs", tag="outt")
                nc.scalar.copy(bbs[:, 0:n], bb[:, 0:n])
                for tb in range(NT):
                    ps = ps_m.tile([128, 512], F32, name="pp", tag="psm")
                    for k in range(ND):
                        nc.tensor.matmul(
                            ps[:, 0:n], h_sb[:, k, 128 * tb:128 * tb + 128],
                            slab[:, k, 0:n],
                            start=(k == 0), stop=(k == ND - 1))
                    ot = op_.tile([128, 512], F32, name="ot", tag="outt")
                    nc.vector.tensor_add(ot[:, 0:n], ps[:, 0:n], bbs[:, 0:n])
                    nc.sync.dma_start(
                        out=out_e[128 * tb:128 * tb + 128,
                                  512 * vs:512 * vs + n],
                        in_=ot[:, 0:n])
    return nc


def _to16(a):
    return np.asarray(a, np.float32).astype(np.float16)


def _cols(v, n):
    Lx = v.shape[0]
    return np.asarray(v, np.float32).reshape(Lx, n, 128).transpose(0, 2, 1)


def prepare_inputs(inputs):
    ids = np.asarray(inputs["input_ids"]).astype(np.int32)
    tok = np.asarray(inputs["tok_emb"], np.float32)
    pos = np.asarray(inputs["pos_emb"], np.float32)[:S]

    par = np.concatenate([
        _cols(inputs["bq"], ND), _cols(inputs["bk"], ND),
        _cols(inputs["bo"], ND), _cols(inputs["b1"], NF),
        _cols(inputs["b2"], ND), _cols(inputs["ln1_g"], ND),
        _cols(inputs["ln1_b"], ND), _cols(inputs["ln2_g"], ND),
        _cols(inputs["ln2_b"], ND)], axis=2).astype(np.float32)
    assert par.shape == (L, 128, NPC)

    fin = np.concatenate([
        np.asarray(inputs["lnf_g"], np.float32).reshape(ND, 128).T,
        np.asarray(inputs["lnf_b"], np.float32).reshape(ND, 128).T],
        axis=1).astype(np.float32)

    shared = {
        "tok_emb": np.ascontiguousarray(tok),
        "Wq": _to16(inputs["Wq"]), "Wk": _to16(inputs["Wk"]),
        "Wv": _to16(inputs["Wv"]), "Wo": _to16(inputs["Wo"]),
        "W1": _to16(inputs["W1"]), "W2": _to16(inputs["W2"]),
        "Wout": _to16(inputs["Wout"]),
        "par": par,
        "bv": np.asarray(inputs["bv"], np.float32).reshape(L, 1, D),
        "fin": fin,
        "bout": np.asarray(inputs["bout"], np.float32).reshape(1, V),
    }

    in_maps = []
    karange = (np.arange(NSLOT)[None, :, None] * 128
               + np.arange(128)[:, None, None])
    for c in range(NCORES):
        b, ch = c // G, c % G
        ids_c = np.ascontiguousarray(
            ids[b, T * ch:T * ch + T].reshape(NT, 128).T)
        pos_c = np.ascontiguousarray(
            pos[T * ch:T * ch + T, :].T.reshape(ND, 128, T).transpose(1, 0, 2))
        qpos = T * ch + np.arange(T)[None, None, :]
        mask_c = (karange <= qpos).astype(ml_dtypes.bfloat16)
        in_maps.append({
            "ids": ids_c, "pos_t": pos_c,
            "masks": np.ascontiguousarray(mask_c), **shared})
    return in_maps


def run(inputs, trace=False):
    if "nc" not in _cache:
        nc = build()
        nc.compile()
        _cache["nc"] = nc
    nc = _cache["nc"]
    in_maps = prepare_inputs(inputs)
    res = run_bass_kernel_spmd(nc, in_maps, core_ids=list(range(NCORES)),
                               trace=trace)
    full = np.empty((B, S, V), np.float32)
    for c in range(NCORES):
        b, ch = c // G, c % G
        full[b, T * ch:T * ch + T, :] = res.results[c]["out"]
    return full, res


def kernel(**inputs):
    full, _ = run(inputs, trace=False)
    return full



# revision 6
# speedup vs baseline: 1.1033x; 1.1033x over previous
"""Distributed 8-layer dense transformer on 8 TRN2 NeuronCores — v2.

Sharding: zigzag context-parallel. Each 4-core group owns one batch
element (1024 tokens = 8 blocks of 128); core c owns blocks (c, 7-c)
("A" and "B" halves, 256 tokens total). This makes causal attention
UNIFORM across cores: A needs key-blocks 0..3, B needs 0..7 (12
block-passes vs 16 for full attention), with per-core masks as data.
All weights replicated; per layer ONE fused K+V AllGather (fp16) per
4-core group. The vocab projection is Megatron-sharded: final h is
AllGathered and each core computes all 1024 group tokens x its own
8000-wide vocab slice (4x less Wout DMA).

Layouts: activations feature-major (x^T: [D, T]). V token-major with
an appended ones-column per head so the softmax denominator rides the
AV matmul. Weights are host-swizzled to [128, k, n] so every slab DMA
is one fat contiguous run per partition.

Attention per head: 8 scores MMs (blocks 0-3 vs all 256 q, blocks 4-7
vs B's 128 q) into 3 PSUM banks; 3 batched exps (Act); mask-mul on the
Pool engine; 8 AV MMs into ONE [65,256] PSUM tile (B-only blocks
accumulate onto cols 128:256); recip+scale on DVE. AV runs one head
behind scores so the PE never waits on Act (keeps the HAM clock gate
released).

PSUM rule: a matmul with start=True clears has_written for its whole
bank, so two interleaved accumulation groups must not share a bank.

Precision: fp16 weights/activations, bf16 exp tiles and V, f32
residual stream / LN stats / PSUM accumulation.
"""

import numpy as np
import ml_dtypes

import concourse.bass as bass
import concourse.mybir as mybir
import concourse.tile as tile
import concourse.bacc as bacc
from concourse.bass_utils import run_bass_kernel_spmd

F32 = mybir.dt.float32
F16 = mybir.dt.float16
BF16 = mybir.dt.bfloat16
I32 = mybir.dt.int32
AF = mybir.ActivationFunctionType
ALU = mybir.AluOpType

L, D, H, DK, F, V, S, B = 8, 1024, 16, 64, 4096, 32000, 1024, 2
NCORES = 8
G = 4
T = 256                 # tokens per core (two 128-blocks: A then B)
NT = T // 128           # 2
ND = D // 128           # 8
NF = F // 128           # 32
NB = 8                  # key blocks of 128 per batch element
VO = DK + 1             # 65
VS = V // G             # 8000 vocab slice per core
NVG = 16                # vocab slabs of 500
VSL = VS // NVG         # 500
EPS = 1e-5
SCALE = 1.0 / np.sqrt(DK)

KVW = ND * 128 + H * VO     # 1024 + 1040 = 2064 fp16 per (block, partition)
EW = 1536                   # e_t / mask width per head

# kv_gath block index for sub-chunk b (AG rank r contributes blocks r, 7-r)
IDX = [0, 2, 4, 6, 7, 5, 3, 1]

PC_BQ, PC_BK, PC_BO, PC_B1, PC_B2 = 0, 8, 16, 24, 56
PC_G1, PC_BE1, PC_G2, PC_BE2 = 64, 72, 80, 88
NPC = 96

_cache = {}


def build():
    nc = bacc.Bacc("TRN2", target_bir_lowering=False, debug=False,
                   num_devices=NCORES)

    ids_e = nc.dram_tensor("ids", [128, NT], I32, kind="ExternalInput")
    tok_e = nc.dram_tensor("tok_emb", [V, D], F32, kind="ExternalInput")
    pos_e = nc.dram_tensor("pos_t", [128, ND, T], F32, kind="ExternalInput")
    mask_e = nc.dram_tensor("masks", [128, EW], BF16, kind="ExternalInput")
    wq_e = nc.dram_tensor("Wq", [L, 2, 128, ND, 512], F16, kind="ExternalInput")
    wk_e = nc.dram_tensor("Wk", [L, 2, 128, ND, 512], F16, kind="ExternalInput")
    wv_e = nc.dram_tensor("Wv", [L, 2, 128, ND, 512], F16, kind="ExternalInput")
    wo_e = nc.dram_tensor("Wo", [L, 2, 128, ND, 512], F16, kind="ExternalInput")
    w1_e = nc.dram_tensor("W1", [L, 8, 128, ND, 512], F16, kind="ExternalInput")
    w2_e = nc.dram_tensor("W2", [L, ND, 128, NF, 128], F16, kind="ExternalInput")
    wout_e = nc.dram_tensor("Wout", [NVG, 128, ND, VSL], F16,
                            kind="ExternalInput")
    par_e = nc.dram_tensor("par", [L, 128, NPC], F32, kind="ExternalInput")
    bv_e = nc.dram_tensor("bv", [L, 1, D], F32, kind="ExternalInput")
    fin_e = nc.dram_tensor("fin", [128, 16], F32, kind="ExternalInput")
    bout_e = nc.dram_tensor("bout", [1, VS], F32, kind="ExternalInput")
    out_e = nc.dram_tensor("out", [NB * 128, VS], F32, kind="ExternalOutput")

    ident_c = nc.inline_tensor(np.eye(128, dtype=np.float32), name="identc")
    ones_c = nc.inline_tensor(np.ones((128, 128), dtype=np.float32), name="onesc")

    with tile.TileContext(nc) as tc:
        with (
            tc.tile_pool(name="persist", bufs=1) as pp,
            tc.tile_pool(name="wp", bufs=4) as wp,
            tc.tile_pool(name="w2p", bufs=2) as w2p,
            tc.tile_pool(name="ep", bufs=3) as ep,
            tc.tile_pool(name="small", bufs=3) as sp,
            tc.tile_pool(name="tmpp", bufs=4) as tp,
            tc.tile_pool(name="bcsp", bufs=2) as bcp,
            tc.tile_pool(name="outp", bufs=4) as op_,
            tc.tile_pool(name="embp", bufs=1) as embp,
            tc.tile_pool(name="ps_m", bufs=4, space="PSUM") as ps_m,
            tc.tile_pool(name="ps_o", bufs=2, space="PSUM") as ps_o,
            tc.tile_pool(name="ps_u", bufs=2, space="PSUM") as ps_u,
            tc.tile_pool(name="dram", bufs=1, space="DRAM") as dp,
        ):
            x_sb = pp.tile([128, ND, T], F32, name="x_sb")
            h_sb = pp.tile([128, ND, T], F16, name="h_sb")
            q_sb = pp.tile([128, ND, T], F16, name="q_sb")
            o_sb = pp.tile([128, ND, T], F16, name="o_sb")
            ktl_sb = pp.tile([128, ND, T], F16, name="ktl_sb")
            vl_sb = pp.tile([128, NT, H * VO], BF16, name="vl_sb")
            kt_sb = pp.tile([128, ND, NB * 128], F16, name="kt_sb")
            v_sb = pp.tile([128, NB, H * VO], BF16, name="v_sb")
            r_sb = pp.tile([128, NF, T], F16, name="r_sb")
            mask_sb = pp.tile([128, EW], BF16, name="mask_sb")
            pos_sb = pp.tile([128, ND, T], F32, name="pos_sb")
            ids_sb = pp.tile([128, NT], I32, name="ids_sb")
            id_sb = pp.tile([128, 128], F32, name="id_sb")
            ones_sb = pp.tile([128, 128], F32, name="ones_sb")
            fin_sb = pp.tile([128, 16], F32, name="fin_sb")
            bvbc_sb = pp.tile([128, D], F32, name="bvbc_sb")
            eps_sb = pp.tile([1, 1], F32, name="eps_sb")

            kv_local = dp.tile([NT, 128, KVW], F16, name="kv_local")
            kv_gath = dp.tile([NB, 128, KVW], F16, name="kv_gath")
            h_local = dp.tile([NT, 128, ND * 128], F16, name="h_local")
            h_gath = dp.tile([NB, 128, ND * 128], F16, name="h_gath")

            nc.sync.dma_start(out=ids_sb[:], in_=ids_e[:])
            nc.sync.dma_start(out=id_sb[:], in_=ident_c[:])
            nc.sync.dma_start(out=ones_sb[:], in_=ones_c[:])
            nc.sync.dma_start(out=pos_sb[:], in_=pos_e[:])
            nc.sync.dma_start(out=mask_sb[:], in_=mask_e[:])
            nc.sync.dma_start(out=fin_sb[:], in_=fin_e[:])
            nc.vector.memset(vl_sb[:], 1.0)
            nc.vector.memset(eps_sb[:], EPS)

            # ---- embedding: gather + transpose to feature-major + pos add
            for tb in range(NT):
                emb = embp.tile([128, D], F32, name="emb")
                nc.gpsimd.indirect_dma_start(
                    out=emb[:], out_offset=None, in_=tok_e[:],
                    in_offset=bass.IndirectOffsetOnAxis(
                        ap=ids_sb[:, tb:tb + 1], axis=0))
                for dt in range(ND):
                    tps = ps_u.tile([128, 512], F32, name="tps", tag="psu")
                    nc.tensor.transpose(
                        tps[:, 0:128], emb[:, 128 * dt:128 * dt + 128], id_sb[:])
                    nc.vector.tensor_add(
                        x_sb[:, dt, 128 * tb:128 * tb + 128],
                        tps[:, 0:128],
                        pos_sb[:, dt, 128 * tb:128 * tb + 128])

            def ln_stats(st1, st2, k):
                """Accumulate sum (st1) and sumsq (st2) of x_sb[:, k, :].
                Separate PSUM banks (start=True clears bank-wide)."""
                nc.tensor.matmul(st1[0:1, 0:T], ones_sb[:, 0:1],
                                 x_sb[:, k, :], start=(k == 0),
                                 stop=(k == ND - 1))
                sq = tp.tile([128, T], F32, name="sq", tag="lntmp")
                nc.scalar.activation(sq[:], x_sb[:, k, :], AF.Square)
                nc.tensor.matmul(st2[0:1, 0:T], ones_sb[:, 0:1],
                                 sq[:], start=(k == 0), stop=(k == ND - 1))

            def ln_apply(st1, st2, par_ap, gcol, bcol, out_sb):
                """Finish LN from stats: x_sb (f32) -> out_sb (f16)."""
                mr = sp.tile([1, 512], F32, name="mr", tag="mr")
                t1 = sp.tile([1, T], F32, name="lns1", tag="lns")
                t2 = sp.tile([1, T], F32, name="lns2", tag="lns")
                nc.scalar.activation(mr[0:1, 0:T], st1[0:1, 0:T], AF.Copy,
                                     scale=1.0 / D)
                nc.scalar.activation(t1[0:1, :], st2[0:1, 0:T], AF.Copy,
                                     scale=1.0 / D)
                nc.vector.tensor_mul(t2[0:1, :], mr[0:1, 0:T], mr[0:1, 0:T])
                nc.vector.tensor_sub(t1[0:1, :], t1[0:1, :], t2[0:1, :])
                nc.scalar.activation(t2[0:1, :], t1[0:1, :], AF.Sqrt,
                                     bias=eps_sb[0:1, 0:1])
                nc.vector.reciprocal(mr[0:1, T:2 * T], t2[0:1, :])
                bc = ps_u.tile([128, 512], F32, name="lnbc", tag="psu")
                nc.tensor.matmul(bc[:, 0:512], ones_sb[0:1, 0:128],
                                 mr[0:1, 0:512], start=True, stop=True)
                bcs = bcp.tile([128, 512], F32, name="bcs", tag="bcs")
                nc.scalar.copy(bcs[:], bc[:])
                for k in range(ND):
                    u1 = tp.tile([128, T], F32, name="u1", tag="lntmp")
                    u2 = tp.tile([128, T], F32, name="u2", tag="lntmp")
                    eng = nc.vector if k % 2 == 0 else nc.gpsimd
                    eng.tensor_sub(u1[:], x_sb[:, k, :], bcs[:, 0:T])
                    eng.tensor_mul(u2[:], u1[:], bcs[:, T:2 * T])
                    eng.tensor_scalar(
                        out=out_sb[:, k, :], in0=u2[:],
                        scalar1=par_ap[:, gcol + k:gcol + k + 1],
                        scalar2=par_ap[:, bcol + k:bcol + k + 1],
                        op0=ALU.mult, op1=ALU.add)

            def std_proj(w_ext, l, dst_sb, bias_par, bias_col):
                """dst[:, m, :] (f16) = (h^T W)[:, m] + bias, feature-major."""
                for c in range(2):
                    slab = wp.tile([128, ND, 512], F16, name="wslab", tag="wslab")
                    nc.sync.dma_start(out=slab[:], in_=w_ext[l, c])
                    for mm in range(4):
                        m = 4 * c + mm
                        ps = ps_m.tile([128, 512], F32, name="pp", tag="psm")
                        for k in range(ND):
                            nc.tensor.matmul(
                                ps[:, 0:T],
                                slab[:, k, 128 * mm:128 * mm + 128],
                                h_sb[:, k, :],
                                start=(k == 0), stop=(k == ND - 1))
                        nc.scalar.activation(
                            dst_sb[:, m, :], ps[:, 0:T], AF.Identity,
                            bias=bias_par[:, bias_col + m:bias_col + m + 1])

            # LN1 stats for layer 0 come from the embedding output
            st1 = ps_u.tile([1, 512], F32, name="st1", tag="psu")
            st2 = ps_u.tile([1, 512], F32, name="st2", tag="psu")
            for k in range(ND):
                ln_stats(st1, st2, k)

            # =================== layers ===================
            for l in range(L):
                par = sp.tile([128, NPC], F32, name="par", tag="par")
                nc.sync.dma_start(out=par[:], in_=par_e[l])
                bv_t = sp.tile([1, D], F32, name="bv_t", tag="bv")
                nc.sync.dma_start(out=bv_t[:], in_=bv_e[l])

                # ---- LN1 (stats were accumulated by the previous loop)
                ln_apply(st1, st2, par, PC_G1, PC_BE1, h_sb)

                # ---- K projection, then V projection, then ONE fused AG
                std_proj(wk_e, l, ktl_sb, par, PC_BK)

                # bv broadcast here: by now ln_apply's bc consumers are done,
                # so the ps_u rotation cannot deadlock across engines
                for c in range(2):
                    bcv = ps_u.tile([128, 512], F32, name="bcv", tag="psu")
                    nc.tensor.matmul(bcv[:], ones_sb[0:1, 0:128],
                                     bv_t[0:1, 512 * c:512 * c + 512],
                                     start=True, stop=True)
                    nc.scalar.copy(bvbc_sb[:, 512 * c:512 * c + 512], bcv[:])
                for tb in range(NT):
                    nc.sync.dma_start(
                        out=kv_local[tb, :, 0:ND * 128].rearrange(
                            "p (k t) -> p k t", k=ND),
                        in_=ktl_sb[:, :, 128 * tb:128 * tb + 128])

                # V projection (token-major, reversed)
                for c in range(2):
                    slab = wp.tile([128, ND, 512], F16, name="wslab", tag="wslab")
                    nc.sync.dma_start(out=slab[:], in_=wv_e[l, c])
                    for tb in range(NT):
                        ps = ps_m.tile([128, 512], F32, name="pp", tag="psm")
                        for k in range(ND):
                            nc.tensor.matmul(
                                ps[:], h_sb[:, k, 128 * tb:128 * tb + 128],
                                slab[:, k, :],
                                start=(k == 0), stop=(k == ND - 1))
                        dst = vl_sb[:, tb,
                                    VO * 8 * c:VO * 8 * c + VO * 8].rearrange(
                            "p (j v) -> p j v", v=VO)[:, :, 0:DK]
                        nc.vector.tensor_add(
                            dst,
                            ps[:].rearrange("p (j v) -> p j v", v=DK),
                            bvbc_sb[:, 512 * c:512 * c + 512].rearrange(
                                "p (j v) -> p j v", v=DK))
                for tb in range(NT):
                    nc.sync.dma_start(
                        out=kv_local[tb, :, ND * 128:KVW],
                        in_=vl_sb[:, tb, :].bitcast(F16))
                nc.gpsimd.collective_compute(
                    "AllGather", ALU.bypass,
                    replica_groups=[[0, 1, 2, 3], [4, 5, 6, 7]],
                    ins=[kv_local[:].opt()], outs=[kv_gath[:].opt()])

                # ---- Q projection (overlaps the AllGather)
                std_proj(wq_e, l, q_sb, par, PC_BQ)

                # ---- pull gathered K^T / V into SBUF (vector's DMA queue so
                #      weight prefetch on sync's queue is not blocked)
                for b in range(NB):
                    nc.scalar.dma_start(
                        out=kt_sb[:, :, 128 * b:128 * b + 128],
                        in_=kv_gath[IDX[b], :, 0:ND * 128].rearrange(
                            "p (k t) -> p k t", k=ND))
                    nc.scalar.dma_start(
                        out=v_sb[:, b, :],
                        in_=kv_gath[IDX[b], :, ND * 128:KVW].bitcast(BF16))

                # ---- attention: 12 block-passes per head, AV one head behind
                def attn_scores(h):
                    po, pt = 64 * (h % 2), h // 2
                    e_t = ep.tile([128, EW], BF16, name="e_t", tag="et")
                    # blocks 0..3 vs all 256 queries -> 2 banks
                    for half in range(2):
                        sa = ps_m.tile([128, 512], F32, name="sa", tag="psm")
                        for i in range(2):
                            b = 2 * half + i
                            nc.tensor.matmul(
                                sa[:, 256 * i:256 * i + 256],
                                kt_sb[po:po + 64, pt, 128 * b:128 * b + 128],
                                q_sb[po:po + 64, pt, :],
                                start=True, stop=True)
                        nc.scalar.activation(
                            e_t[:, 512 * half:512 * half + 512], sa[:], AF.Exp,
                            scale=float(SCALE))
                    # blocks 4..7 vs B queries only -> 1 bank
                    sa = ps_m.tile([128, 512], F32, name="sa", tag="psm")
                    for i in range(4):
                        b = 4 + i
                        nc.tensor.matmul(
                            sa[:, 128 * i:128 * i + 128],
                            kt_sb[po:po + 64, pt, 128 * b:128 * b + 128],
                            q_sb[po:po + 64, pt, 128:256],
                            start=True, stop=True)
                    nc.scalar.activation(
                        e_t[:, 1024:1536], sa[:], AF.Exp, scale=float(SCALE))
                    nc.gpsimd.tensor_mul(e_t[:, 0:1024], e_t[:, 0:1024],
                                         mask_sb[:, 0:1024])
                    nc.gpsimd.tensor_mul(e_t[:, 1024:1536], e_t[:, 1024:1536],
                                         mask_sb[:, 1024:1536])
                    return e_t

                def attn_av(h, e_t):
                    po, pt = 64 * (h % 2), h // 2
                    oo = ps_o.tile([VO, T], F32, name="oo", tag="pso")
                    for b in range(4):
                        nc.tensor.matmul(
                            oo[:], v_sb[:, b, VO * h:VO * h + VO],
                            e_t[:, 256 * b:256 * b + 256],
                            start=(b == 0), stop=False)
                    for b in range(4):
                        nc.tensor.matmul(
                            oo[:, 128:256], v_sb[:, 4 + b, VO * h:VO * h + VO],
                            e_t[:, 1024 + 128 * b:1152 + 128 * b],
                            start=False, stop=(b == 3))
                    rec = sp.tile([1, T], F32, name="rec", tag="rec")
                    nc.vector.reciprocal(rec[0:1, :], oo[DK:VO, :])
                    rbc = ps_u.tile([128, 512], F32, name="rbc", tag="psu")
                    nc.tensor.matmul(rbc[0:64, 0:T], ones_sb[0:1, 0:64],
                                     rec[0:1, :], start=True, stop=True)
                    rbs = tp.tile([64, T], F32, name="rbs", tag="rbs")
                    nc.vector.tensor_copy(out=rbs[:], in_=rbc[0:64, 0:T])
                    nc.vector.tensor_mul(o_sb[po:po + 64, pt, :],
                                         oo[0:DK, :], rbs[:])

                prev = None
                for h in range(H):
                    e_t = attn_scores(h)
                    if prev is not None:
                        attn_av(prev[0], prev[1])
                    prev = (h, e_t)
                attn_av(prev[0], prev[1])

                # ---- attention output projection + residual + LN2 stats
                st1 = ps_u.tile([1, 512], F32, name="st1", tag="psu")
                st2 = ps_u.tile([1, 512], F32, name="st2", tag="psu")
                for c in range(2):
                    slab = wp.tile([128, ND, 512], F16, name="wslab", tag="wslab")
                    nc.sync.dma_start(out=slab[:], in_=wo_e[l, c])
                    for mm in range(4):
                        m = 4 * c + mm
                        ps = ps_m.tile([128, 512], F32, name="pp", tag="psm")
                        for k in range(ND):
                            nc.tensor.matmul(
                                ps[:, 0:T],
                                slab[:, k, 128 * mm:128 * mm + 128],
                                o_sb[:, k, :],
                                start=(k == 0), stop=(k == ND - 1))
                        rt = tp.tile([128, T], F32, name="rt", tag="lntmp")
                        nc.scalar.activation(
                            rt[:], ps[:, 0:T], AF.Identity,
                            bias=par[:, PC_BO + m:PC_BO + m + 1])
                        nc.vector.tensor_add(x_sb[:, m, :], x_sb[:, m, :], rt[:])
                        ln_stats(st1, st2, m)

                # ---- LN2
                ln_apply(st1, st2, par, PC_G2, PC_BE2, h_sb)

                # ---- FFN W1 + relu
                for c in range(8):
                    slab = wp.tile([128, ND, 512], F16, name="wslab", tag="wslab")
                    nc.sync.dma_start(out=slab[:], in_=w1_e[l, c])
                    for mm in range(4):
                        ot = 4 * c + mm
                        ps = ps_m.tile([128, 512], F32, name="pp", tag="psm")
                        for k in range(ND):
                            nc.tensor.matmul(
                                ps[:, 0:T],
                                slab[:, k, 128 * mm:128 * mm + 128],
                                h_sb[:, k, :],
                                start=(k == 0), stop=(k == ND - 1))
                        nc.scalar.activation(
                            r_sb[:, ot, :], ps[:, 0:T], AF.Relu,
                            bias=par[:, PC_B1 + ot:PC_B1 + ot + 1])

                # ---- FFN W2 + residual + next-LN stats
                st1 = ps_u.tile([1, 512], F32, name="st1", tag="psu")
                st2 = ps_u.tile([1, 512], F32, name="st2", tag="psu")
                for m in range(ND):
                    slab2 = w2p.tile([128, NF, 128], F16, name="w2slab",
                                     tag="w2slab")
                    nc.sync.dma_start(out=slab2[:], in_=w2_e[l, m])
                    ps = ps_m.tile([128, 512], F32, name="pp", tag="psm")
                    for k in range(NF):
                        nc.tensor.matmul(
                            ps[:, 0:T], slab2[:, k, :], r_sb[:, k, :],
                            start=(k == 0), stop=(k == NF - 1))
                    rt = tp.tile([128, T], F32, name="rt2", tag="lntmp")
                    nc.scalar.activation(
                        rt[:], ps[:, 0:T], AF.Identity,
                        bias=par[:, PC_B2 + m:PC_B2 + m + 1])
                    nc.vector.tensor_add(x_sb[:, m, :], x_sb[:, m, :], rt[:])
                    ln_stats(st1, st2, m)

            # =================== final LN + sharded vocab projection ========
            ln_apply(st1, st2, fin_sb, 0, 8, h_sb)
            for tb in range(NT):
                nc.sync.dma_start(
                    out=h_local[tb].rearrange("p (k t) -> p k t", k=ND),
                    in_=h_sb[:, :, 128 * tb:128 * tb + 128])
            nc.gpsimd.collective_compute(
                "AllGather", ALU.bypass,
                replica_groups=[[0, 1, 2, 3], [4, 5, 6, 7]],
                ins=[h_local[:].opt()], outs=[h_gath[:].opt()])
            # gathered h for all 1024 group tokens, canonical block order
            for b in range(NB):
                nc.scalar.dma_start(
                    out=kt_sb[:, :, 128 * b:128 * b + 128],
                    in_=h_gath[IDX[b]].rearrange("p (k t) -> p k t", k=ND))

            for vs in range(NVG):
                slab = wp.tile([128, ND, 512], F16, name="wslab", tag="wslab")
                nc.sync.dma_start(out=slab[:, :, 0:VSL], in_=wout_e[vs])
                bo_t = sp.tile([1, 512], F32, name="bo_t", tag="bo")
                nc.sync.dma_start(out=bo_t[0:1, 0:VSL],
                                  in_=bout_e[0:1, VSL * vs:VSL * vs + VSL])
                bb = ps_u.tile([128, 512], F32, name="bb", tag="psu")
                nc.tensor.matmul(bb[:, 0:VSL], ones_sb[0:1, 0:128],
                                 bo_t[0:1, 0:VSL], start=True, stop=True)
                bbs = op_.tile([128, 512], F32, name="bbs", tag="outt")
                nc.scalar.copy(bbs[:, 0:VSL], bb[:, 0:VSL])
                for tb in range(NB):
                    ps = ps_m.tile([128, 512], F32, name="pp", tag="psm")
                    for k in range(ND):
                        nc.tensor.matmul(
                            ps[:, 0:VSL], kt_sb[:, k, 128 * tb:128 * tb + 128],
                            slab[:, k, 0:VSL],
                            start=(k == 0), stop=(k == ND - 1))
                    ot = op_.tile([128, 512], F32, name="ot", tag="outt")
                    nc.vector.tensor_add(ot[:, 0:VSL], ps[:, 0:VSL],
                                         bbs[:, 0:VSL])
                    nc.sync.dma_start(
                        out=out_e[128 * tb:128 * tb + 128,
                                  VSL * vs:VSL * vs + VSL],
                        in_=ot[:, 0:VSL])
    return nc


def _to16(a):
    return np.asarray(a, np.float32).astype(np.float16)


def _cols(v, n):
    Lx = v.shape[0]
    return np.asarray(v, np.float32).reshape(Lx, n, 128).transpose(0, 2, 1)


def _sw(w, nslab, width):
    """[..., D_in, N] row-major -> [..., nslab, 128, D_in//128, width]."""
    lead = w.shape[:-2]
    din, n = w.shape[-2], w.shape[-1]
    k = din // 128
    assert n == nslab * width
    w = w.reshape(*lead, k, 128, nslab, width)
    order = tuple(range(len(lead))) + tuple(
        len(lead) + i for i in (2, 1, 0, 3))
    return np.ascontiguousarray(w.transpose(order))


def prepare_inputs(inputs):
    ids = np.asarray(inputs["input_ids"]).astype(np.int32)
    pos = np.asarray(inputs["pos_emb"], np.float32)[:S]

    par = np.concatenate([
        _cols(inputs["bq"], ND), _cols(inputs["bk"], ND),
        _cols(inputs["bo"], ND), _cols(inputs["b1"], NF),
        _cols(inputs["b2"], ND), _cols(inputs["ln1_g"], ND),
        _cols(inputs["ln1_b"], ND), _cols(inputs["ln2_g"], ND),
        _cols(inputs["ln2_b"], ND)], axis=2).astype(np.float32)
    assert par.shape == (L, 128, NPC)

    fin = np.concatenate([
        np.asarray(inputs["lnf_g"], np.float32).reshape(ND, 128).T,
        np.asarray(inputs["lnf_b"], np.float32).reshape(ND, 128).T],
        axis=1).astype(np.float32)

    wout16 = _to16(inputs["Wout"])
    bout = np.asarray(inputs["bout"], np.float32)

    shared = {
        "tok_emb": np.ascontiguousarray(np.asarray(inputs["tok_emb"],
                                                   np.float32)),
        "Wq": _sw(_to16(inputs["Wq"]), 2, 512),
        "Wk": _sw(_to16(inputs["Wk"]), 2, 512),
        "Wv": _sw(_to16(inputs["Wv"]), 2, 512),
        "Wo": _sw(_to16(inputs["Wo"]), 2, 512),
        "W1": _sw(_to16(inputs["W1"]), 8, 512),
        "W2": _sw(_to16(inputs["W2"]), ND, 128),
        "par": par,
        "bv": np.asarray(inputs["bv"], np.float32).reshape(L, 1, D),
        "fin": fin,
    }

    in_maps = []
    kk = np.arange(128)[:, None]                      # key within block
    qq = np.arange(128)[None, :]                      # query within block
    for c in range(NCORES):
        b, ch = c // G, c % G
        a_blk, b_blk = ch, 7 - ch
        tokA = slice(128 * a_blk, 128 * a_blk + 128)
        tokB = slice(128 * b_blk, 128 * b_blk + 128)
        ids_c = np.ascontiguousarray(
            np.stack([ids[b, tokA], ids[b, tokB]], axis=1))
        pos_c = np.ascontiguousarray(
            np.concatenate([pos[tokA], pos[tokB]], axis=0)
            .T.reshape(ND, 128, T).transpose(1, 0, 2))
        # masks [128 keys, 1536]: blocks 0..3 x [A|B] queries, 4..7 x B
        m = np.zeros((128, EW), np.float32)
        for blk in range(4):
            kg = 128 * blk + kk
            m[:, 256 * blk:256 * blk + 128] = kg <= (128 * a_blk + qq)
            m[:, 256 * blk + 128:256 * blk + 256] = kg <= (128 * b_blk + qq)
        for blk in range(4, 8):
            kg = 128 * blk + kk
            m[:, 1024 + 128 * (blk - 4):1152 + 128 * (blk - 4)] = \
                kg <= (128 * b_blk + qq)
        wout_c = _sw(wout16[:, VS * ch:VS * ch + VS], NVG, VSL)
        in_maps.append({
            "ids": ids_c, "pos_t": pos_c,
            "masks": np.ascontiguousarray(m.astype(ml_dtypes.bfloat16)),
            "Wout": wout_c,
            "bout": np.ascontiguousarray(bout[VS * ch:VS * ch + VS]
                                         .reshape(1, VS)),
            **shared})
    return in_maps


def run(inputs, trace=False):
    if "nc" not in _cache:
        nc = build()
        nc.compile()
        _cache["nc"] = nc
    nc = _cache["nc"]
    in_maps = prepare_inputs(inputs)
    res = run_bass_kernel_spmd(nc, in_maps, core_ids=list(range(NCORES)),
                               trace=trace)
    full = np.empty((B, S, V), np.float32)
    for c in range(NCORES):
        b, ch = c // G, c % G
        full[b, :, VS * ch:VS * ch + VS] = res.results[c]["out"]
    return full, res


def kernel(**inputs):
    full, _ = run(inputs, trace=False)
    return full


# revision 10
# speedup vs baseline: 1.1391x; 1.0325x over previous
"""Distributed 8-layer dense transformer on 8 TRN2 NeuronCores — v2.

Sharding: zigzag context-parallel. Each 4-core group owns one batch
element (1024 tokens = 8 blocks of 128); core c owns blocks (c, 7-c)
("A" and "B" halves, 256 tokens total). This makes causal attention
UNIFORM across cores: A needs key-blocks 0..3, B needs 0..7 (12
block-passes vs 16 for full attention), with per-core masks as data.
All weights replicated; per layer ONE fused K+V AllGather (fp16) per
4-core group. The vocab projection is Megatron-sharded: final h is
AllGathered and each core computes all 1024 group tokens x its own
8000-wide vocab slice (4x less Wout DMA).

Layouts: activations feature-major (x^T: [D, T]). V token-major with
an appended ones-column per head so the softmax denominator rides the
AV matmul. Weights are host-swizzled to [128, k, n] so every slab DMA
is one fat contiguous run per partition.

Attention per head: 8 scores MMs (blocks 0-3 vs all 256 q, blocks 4-7
vs B's 128 q) into 3 PSUM banks; 3 batched exps (Act); mask-mul on the
Pool engine; 8 AV MMs into ONE [65,256] PSUM tile (B-only blocks
accumulate onto cols 128:256); recip+scale on DVE. AV runs one head
behind scores so the PE never waits on Act (keeps the HAM clock gate
released).

PSUM rule: a matmul with start=True clears has_written for its whole
bank, so two interleaved accumulation groups must not share a bank.

Precision: fp16 weights/activations, bf16 exp tiles and V, f32
residual stream / LN stats / PSUM accumulation.
"""

import numpy as np
import ml_dtypes

import concourse.bass as bass
import concourse.mybir as mybir
import concourse.tile as tile
import concourse.bacc as bacc
from concourse.bass_utils import run_bass_kernel_spmd

F32 = mybir.dt.float32
F16 = mybir.dt.float16
BF16 = mybir.dt.bfloat16
I32 = mybir.dt.int32
AF = mybir.ActivationFunctionType
ALU = mybir.AluOpType

L, D, H, DK, F, V, S, B = 8, 1024, 16, 64, 4096, 32000, 1024, 2
NCORES = 8
G = 4
T = 256                 # tokens per core (two 128-blocks: A then B)
NT = T // 128           # 2
ND = D // 128           # 8
NF = F // 128           # 32
NB = 8                  # key blocks of 128 per batch element
VO = DK + 1             # 65
VS = V // G             # 8000 vocab slice per core
NVG = 16                # vocab slabs of 500
VSL = VS // NVG         # 500
EPS = 1e-5
SCALE = 1.0 / np.sqrt(DK)

KVW = ND * 128 + H * VO     # 1024 + 1040 = 2064 fp16 per (block, partition)
EW = 1536                   # e_t / mask width per head

# gather position of canonical block b (AG rank r contributes blocks r, 7-r)
GP = [0, 2, 4, 6, 7, 5, 3, 1]
CAN = [0, 7, 1, 6, 2, 5, 3, 4]   # canonical block held at gather position g

PC_BQ, PC_BK, PC_BO, PC_B1, PC_B2 = 0, 8, 16, 24, 56
PC_G1, PC_BE1, PC_G2, PC_BE2 = 64, 72, 80, 88
NPC = 96

_cache = {}


def build():
    nc = bacc.Bacc("TRN2", target_bir_lowering=False, debug=False,
                   num_devices=NCORES)

    ids_e = nc.dram_tensor("ids", [128, NT], I32, kind="ExternalInput")
    tok_e = nc.dram_tensor("tok_emb", [V, D], F32, kind="ExternalInput")
    pos_e = nc.dram_tensor("pos_t", [128, ND, T], F32, kind="ExternalInput")
    mask_e = nc.dram_tensor("masks", [128, EW], BF16, kind="ExternalInput")
    wq_e = nc.dram_tensor("Wq", [L, 2, 128, ND, 512], F16, kind="ExternalInput")
    wk_e = nc.dram_tensor("Wk", [L, 2, 128, ND, 512], F16, kind="ExternalInput")
    wv_e = nc.dram_tensor("Wv", [L, 2, 128, ND, 512], F16, kind="ExternalInput")
    wo_e = nc.dram_tensor("Wo", [L, 2, 128, ND, 512], F16, kind="ExternalInput")
    w1_e = nc.dram_tensor("W1", [L, 8, 128, ND, 512], F16, kind="ExternalInput")
    w2_e = nc.dram_tensor("W2", [L, ND, 128, NF, 128], F16, kind="ExternalInput")
    wout_e = nc.dram_tensor("Wout", [NVG, 128, ND, VSL], F16,
                            kind="ExternalInput")
    par_e = nc.dram_tensor("par", [L, 128, NPC], F32, kind="ExternalInput")
    bv_e = nc.dram_tensor("bv", [L, 1, D], F32, kind="ExternalInput")
    fin_e = nc.dram_tensor("fin", [128, 16], F32, kind="ExternalInput")
    bout_e = nc.dram_tensor("bout", [1, VS], F32, kind="ExternalInput")
    out_e = nc.dram_tensor("out", [NB * 128, VS], F32, kind="ExternalOutput")

    ident_c = nc.inline_tensor(np.eye(128, dtype=np.float32), name="identc")
    ones_c = nc.inline_tensor(np.ones((128, 128), dtype=np.float32), name="onesc")

    with tile.TileContext(nc) as tc:
        with (
            tc.tile_pool(name="persist", bufs=1) as pp,
            tc.tile_pool(name="wp", bufs=4) as wp,
            tc.tile_pool(name="w2p", bufs=2) as w2p,
            tc.tile_pool(name="ep", bufs=4) as ep,
            tc.tile_pool(name="small", bufs=3) as sp,
            tc.tile_pool(name="tmpp", bufs=4) as tp,
            tc.tile_pool(name="bcsp", bufs=2) as bcp,
            tc.tile_pool(name="outp", bufs=4) as op_,
            tc.tile_pool(name="embp", bufs=1) as embp,
            tc.tile_pool(name="ps_m", bufs=4, space="PSUM") as ps_m,
            tc.tile_pool(name="ps_o", bufs=2, space="PSUM") as ps_o,
            tc.tile_pool(name="ps_u", bufs=2, space="PSUM") as ps_u,
            tc.tile_pool(name="dram", bufs=1, space="DRAM") as dp,
        ):
            x_sb = pp.tile([128, ND, T], F32, name="x_sb")
            h_sb = pp.tile([128, ND, T], F16, name="h_sb")
            q_sb = pp.tile([128, ND, T], F16, name="q_sb")
            o_sb = pp.tile([128, ND, T], F16, name="o_sb")
            ktl_sb = pp.tile([128, ND, T], F16, name="ktl_sb")
            vl_sb = pp.tile([128, NT, H * VO], BF16, name="vl_sb")
            kt_sb = pp.tile([128, NB, ND, 128], F16, name="kt_sb")
            v_sb = pp.tile([128, NB, H * VO], BF16, name="v_sb")
            r_sb = pp.tile([128, NF, T], F16, name="r_sb")
            mask_sb = pp.tile([128, EW], BF16, name="mask_sb")
            pos_sb = pp.tile([128, ND, T], F32, name="pos_sb")
            ids_sb = pp.tile([128, NT], I32, name="ids_sb")
            id_sb = pp.tile([128, 128], F32, name="id_sb")
            ones_sb = pp.tile([128, 128], F32, name="ones_sb")
            fin_sb = pp.tile([128, 16], F32, name="fin_sb")
            bvbc_sb = pp.tile([128, D], F32, name="bvbc_sb")
            eps_sb = pp.tile([1, 1], F32, name="eps_sb")

            k_local = dp.tile([NT, 128, ND * 128], F16, name="k_local")
            k_gath = dp.tile([NB, 128, ND * 128], F16, name="k_gath")
            v_local = dp.tile([NT, 128, H * VO], F16, name="v_local")
            v_gath = dp.tile([NB, 128, H * VO], F16, name="v_gath")
            h_local = dp.tile([NT, 128, ND * 128], F16, name="h_local")
            h_gath = dp.tile([NB, 128, ND * 128], F16, name="h_gath")

            nc.sync.dma_start(out=ids_sb[:], in_=ids_e[:])
            nc.sync.dma_start(out=id_sb[:], in_=ident_c[:])
            nc.sync.dma_start(out=ones_sb[:], in_=ones_c[:])
            nc.sync.dma_start(out=pos_sb[:], in_=pos_e[:])
            nc.sync.dma_start(out=mask_sb[:], in_=mask_e[:])
            nc.sync.dma_start(out=fin_sb[:], in_=fin_e[:])
            nc.vector.memset(vl_sb[:], 1.0)
            nc.vector.memset(eps_sb[:], EPS)

            # ---- embedding: gather + transpose to feature-major + pos add
            for tb in range(NT):
                emb = embp.tile([128, D], F32, name="emb")
                nc.gpsimd.indirect_dma_start(
                    out=emb[:], out_offset=None, in_=tok_e[:],
                    in_offset=bass.IndirectOffsetOnAxis(
                        ap=ids_sb[:, tb:tb + 1], axis=0))
                for dt in range(ND):
                    tps = ps_u.tile([128, 512], F32, name="tps", tag="psu")
                    nc.tensor.transpose(
                        tps[:, 0:128], emb[:, 128 * dt:128 * dt + 128], id_sb[:])
                    nc.vector.tensor_add(
                        x_sb[:, dt, 128 * tb:128 * tb + 128],
                        tps[:, 0:128],
                        pos_sb[:, dt, 128 * tb:128 * tb + 128])

            def ln_stats(st1, st2, k):
                """Accumulate sum (st1) and sumsq (st2) of x_sb[:, k, :].
                Separate PSUM banks (start=True clears bank-wide)."""
                nc.tensor.matmul(st1[0:1, 0:T], ones_sb[:, 0:1],
                                 x_sb[:, k, :], start=(k == 0),
                                 stop=(k == ND - 1))
                sq = tp.tile([128, T], F32, name="sq", tag="lntmp")
                nc.scalar.activation(sq[:], x_sb[:, k, :], AF.Square)
                nc.tensor.matmul(st2[0:1, 0:T], ones_sb[:, 0:1],
                                 sq[:], start=(k == 0), stop=(k == ND - 1))

            def ln_apply(st1, st2, par_ap, gcol, bcol, out_sb):
                """Finish LN from stats: x_sb (f32) -> out_sb (f16)."""
                mr = sp.tile([1, 512], F32, name="mr", tag="mr")
                t1 = sp.tile([1, T], F32, name="lns1", tag="lns")
                t2 = sp.tile([1, T], F32, name="lns2", tag="lns")
                nc.scalar.activation(mr[0:1, 0:T], st1[0:1, 0:T], AF.Copy,
                                     scale=1.0 / D)
                nc.scalar.activation(t1[0:1, :], st2[0:1, 0:T], AF.Copy,
                                     scale=1.0 / D)
                nc.vector.tensor_mul(t2[0:1, :], mr[0:1, 0:T], mr[0:1, 0:T])
                nc.vector.tensor_sub(t1[0:1, :], t1[0:1, :], t2[0:1, :])
                nc.scalar.activation(t2[0:1, :], t1[0:1, :], AF.Sqrt,
                                     bias=eps_sb[0:1, 0:1])
                nc.vector.reciprocal(mr[0:1, T:2 * T], t2[0:1, :])
                bc = ps_u.tile([128, 512], F32, name="lnbc", tag="psu")
                nc.tensor.matmul(bc[:, 0:512], ones_sb[0:1, 0:128],
                                 mr[0:1, 0:512], start=True, stop=True)
                bcs = bcp.tile([128, 512], F32, name="bcs", tag="bcs")
                nc.scalar.copy(bcs[:], bc[:])
                for k in range(ND):
                    u1 = tp.tile([128, T], F32, name="u1", tag="lntmp")
                    u2 = tp.tile([128, T], F32, name="u2", tag="lntmp")
                    eng = nc.vector if k % 2 == 0 else nc.gpsimd
                    eng.tensor_sub(u1[:], x_sb[:, k, :], bcs[:, 0:T])
                    eng.tensor_mul(u2[:], u1[:], bcs[:, T:2 * T])
                    eng.tensor_scalar(
                        out=out_sb[:, k, :], in0=u2[:],
                        scalar1=par_ap[:, gcol + k:gcol + k + 1],
                        scalar2=par_ap[:, bcol + k:bcol + k + 1],
                        op0=ALU.mult, op1=ALU.add)

            def std_proj(w_ext, l, dst_sb, bias_par, bias_col):
                """dst[:, m, :] (f16) = (h^T W)[:, m] + bias, feature-major."""
                for c in range(2):
                    slab = wp.tile([128, ND, 512], F16, name="wslab", tag="wslab")
                    nc.sync.dma_start(out=slab[:], in_=w_ext[l, c])
                    for mm in range(4):
                        m = 4 * c + mm
                        ps = ps_m.tile([128, 512], F32, name="pp", tag="psm")
                        for k in range(ND):
                            nc.tensor.matmul(
                                ps[:, 0:T],
                                slab[:, k, 128 * mm:128 * mm + 128],
                                h_sb[:, k, :],
                                start=(k == 0), stop=(k == ND - 1))
                        nc.scalar.activation(
                            dst_sb[:, m, :], ps[:, 0:T], AF.Identity,
                            bias=bias_par[:, bias_col + m:bias_col + m + 1])

            # LN1 stats for layer 0 come from the embedding output
            st1 = ps_u.tile([1, 512], F32, name="st1", tag="psu")
            st2 = ps_u.tile([1, 512], F32, name="st2", tag="psu")
            for k in range(ND):
                ln_stats(st1, st2, k)

            # =================== layers ===================
            for l in range(L):
                par = sp.tile([128, NPC], F32, name="par", tag="par")
                nc.sync.dma_start(out=par[:], in_=par_e[l])
                bv_t = sp.tile([1, D], F32, name="bv_t", tag="bv")
                nc.sync.dma_start(out=bv_t[:], in_=bv_e[l])

                # ---- LN1 (stats were accumulated by the previous loop)
                ln_apply(st1, st2, par, PC_G1, PC_BE1, h_sb)

                # ---- K projection, then V projection, then ONE fused AG
                std_proj(wk_e, l, ktl_sb, par, PC_BK)

                # bv broadcast here: by now ln_apply's bc consumers are done,
                # so the ps_u rotation cannot deadlock across engines
                for c in range(2):
                    bcv = ps_u.tile([128, 512], F32, name="bcv", tag="psu")
                    nc.tensor.matmul(bcv[:], ones_sb[0:1, 0:128],
                                     bv_t[0:1, 512 * c:512 * c + 512],
                                     start=True, stop=True)
                    nc.scalar.copy(bvbc_sb[:, 512 * c:512 * c + 512], bcv[:])
                for tb in range(NT):
                    nc.sync.dma_start(
                        out=k_local[tb].rearrange("p (k t) -> p k t", k=ND),
                        in_=ktl_sb[:, :, 128 * tb:128 * tb + 128])
                nc.gpsimd.collective_compute(
                    "AllGather", ALU.bypass,
                    replica_groups=[[0, 1, 2, 3], [4, 5, 6, 7]],
                    ins=[k_local[:].opt()], outs=[k_gath[:].opt()])

                # V projection (token-major, reversed)
                for c in range(2):
                    slab = wp.tile([128, ND, 512], F16, name="wslab", tag="wslab")
                    nc.sync.dma_start(out=slab[:], in_=wv_e[l, c])
                    for tb in range(NT):
                        ps = ps_m.tile([128, 512], F32, name="pp", tag="psm")
                        for k in range(ND):
                            nc.tensor.matmul(
                                ps[:], h_sb[:, k, 128 * tb:128 * tb + 128],
                                slab[:, k, :],
                                start=(k == 0), stop=(k == ND - 1))
                        dst = vl_sb[:, tb,
                                    VO * 8 * c:VO * 8 * c + VO * 8].rearrange(
                            "p (j v) -> p j v", v=VO)[:, :, 0:DK]
                        nc.vector.tensor_add(
                            dst,
                            ps[:].rearrange("p (j v) -> p j v", v=DK),
                            bvbc_sb[:, 512 * c:512 * c + 512].rearrange(
                                "p (j v) -> p j v", v=DK))
                for tb in range(NT):
                    nc.sync.dma_start(
                        out=v_local[tb],
                        in_=vl_sb[:, tb, :].bitcast(F16))
                nc.gpsimd.collective_compute(
                    "AllGather", ALU.bypass,
                    replica_groups=[[0, 1, 2, 3], [4, 5, 6, 7]],
                    ins=[v_local[:].opt()], outs=[v_gath[:].opt()])

                # ---- Q projection (overlaps the AllGather)
                std_proj(wq_e, l, q_sb, par, PC_BQ)

                # ---- pull gathered K^T / V into SBUF: ONE dma each, on the
                #      Act engine's queue (sync keeps prefetching weights).
                #      SBUF holds blocks in GATHER order; loops index via GP.
                nc.scalar.dma_start(
                    out=kt_sb[:].rearrange("p g k t -> p g (k t)"),
                    in_=k_gath[:].rearrange("g p c -> p g c"))
                nc.scalar.dma_start(
                    out=v_sb[:].bitcast(F16),
                    in_=v_gath[:].rearrange("g p c -> p g c"))

                # ---- attention: 12 block-passes per head, AV one head behind
                def attn_scores(h):
                    po, pt = 64 * (h % 2), h // 2
                    e_t = ep.tile([128, EW], BF16, name="e_t", tag="et")
                    # blocks 0..3 vs all 256 queries -> 2 banks
                    for half in range(2):
                        sa = ps_m.tile([128, 512], F32, name="sa", tag="psm")
                        for i in range(2):
                            b = 2 * half + i
                            nc.tensor.matmul(
                                sa[:, 256 * i:256 * i + 256],
                                kt_sb[po:po + 64, GP[b], pt, :],
                                q_sb[po:po + 64, pt, :],
                                start=True, stop=True)
                        nc.scalar.activation(
                            e_t[:, 512 * half:512 * half + 512], sa[:], AF.Exp,
                            scale=float(SCALE))
                        nc.vector.tensor_mul(
                            e_t[:, 512 * half:512 * half + 512],
                            e_t[:, 512 * half:512 * half + 512],
                            mask_sb[:, 512 * half:512 * half + 512])
                    # blocks 4..7 vs B queries only -> 1 bank
                    sa = ps_m.tile([128, 512], F32, name="sa", tag="psm")
                    for i in range(4):
                        b = 4 + i
                        nc.tensor.matmul(
                            sa[:, 128 * i:128 * i + 128],
                            kt_sb[po:po + 64, GP[b], pt, :],
                            q_sb[po:po + 64, pt, 128:256],
                            start=True, stop=True)
                    nc.scalar.activation(
                        e_t[:, 1024:1536], sa[:], AF.Exp, scale=float(SCALE))
                    nc.gpsimd.tensor_mul(e_t[:, 1024:1536], e_t[:, 1024:1536],
                                         mask_sb[:, 1024:1536])
                    return e_t

                def attn_av(h, e_t):
                    po, pt = 64 * (h % 2), h // 2
                    oo = ps_o.tile([VO, T], F32, name="oo", tag="pso")
                    for b in range(4):
                        nc.tensor.matmul(
                            oo[:], v_sb[:, GP[b], VO * h:VO * h + VO],
                            e_t[:, 256 * b:256 * b + 256],
                            start=(b == 0), stop=False)
                    for b in range(4):
                        nc.tensor.matmul(
                            oo[:, 128:256], v_sb[:, GP[4 + b], VO * h:VO * h + VO],
                            e_t[:, 1024 + 128 * b:1152 + 128 * b],
                            start=False, stop=(b == 3))
                    rec = sp.tile([1, T], F32, name="rec", tag="rec")
                    nc.vector.reciprocal(rec[0:1, :], oo[DK:VO, :])
                    rbc = ps_u.tile([128, 512], F32, name="rbc", tag="psu")
                    nc.tensor.matmul(rbc[0:64, 0:T], ones_sb[0:1, 0:64],
                                     rec[0:1, :], start=True, stop=True)
                    rbs = tp.tile([64, T], F32, name="rbs", tag="rbs")
                    nc.vector.tensor_copy(out=rbs[:], in_=rbc[0:64, 0:T])
                    nc.vector.tensor_mul(o_sb[po:po + 64, pt, :],
                                         oo[0:DK, :], rbs[:])

                pend = []
                for h in range(H):
                    e_t = attn_scores(h)
                    pend.append((h, e_t))
                    if len(pend) > 2:
                        attn_av(*pend.pop(0))
                for pe_ in pend:
                    attn_av(*pe_)

                # ---- attention output projection + residual + LN2 stats
                st1 = ps_u.tile([1, 512], F32, name="st1", tag="psu")
                st2 = ps_u.tile([1, 512], F32, name="st2", tag="psu")
                for c in range(2):
                    slab = wp.tile([128, ND, 512], F16, name="wslab", tag="wslab")
                    nc.sync.dma_start(out=slab[:], in_=wo_e[l, c])
                    for mm in range(4):
                        m = 4 * c + mm
                        ps = ps_m.tile([128, 512], F32, name="pp", tag="psm")
                        for k in range(ND):
                            nc.tensor.matmul(
                                ps[:, 0:T],
                                slab[:, k, 128 * mm:128 * mm + 128],
                                o_sb[:, k, :],
                                start=(k == 0), stop=(k == ND - 1))
                        rt = tp.tile([128, T], F32, name="rt", tag="lntmp")
                        nc.scalar.activation(
                            rt[:], ps[:, 0:T], AF.Identity,
                            bias=par[:, PC_BO + m:PC_BO + m + 1])
                        nc.vector.tensor_add(x_sb[:, m, :], x_sb[:, m, :], rt[:])
                        ln_stats(st1, st2, m)

                # ---- LN2
                ln_apply(st1, st2, par, PC_G2, PC_BE2, h_sb)

                # ---- FFN W1 + relu
                for c in range(8):
                    slab = wp.tile([128, ND, 512], F16, name="wslab", tag="wslab")
                    nc.sync.dma_start(out=slab[:], in_=w1_e[l, c])
                    for mm in range(4):
                        ot = 4 * c + mm
                        ps = ps_m.tile([128, 512], F32, name="pp", tag="psm")
                        for k in range(ND):
                            nc.tensor.matmul(
                                ps[:, 0:T],
                                slab[:, k, 128 * mm:128 * mm + 128],
                                h_sb[:, k, :],
                                start=(k == 0), stop=(k == ND - 1))
                        nc.scalar.activation(
                            r_sb[:, ot, :], ps[:, 0:T], AF.Relu,
                            bias=par[:, PC_B1 + ot:PC_B1 + ot + 1])

                # ---- FFN W2 + residual + next-LN stats
                st1 = ps_u.tile([1, 512], F32, name="st1", tag="psu")
                st2 = ps_u.tile([1, 512], F32, name="st2", tag="psu")
                for m in range(ND):
                    slab2 = w2p.tile([128, NF, 128], F16, name="w2slab",
                                     tag="w2slab")
                    nc.sync.dma_start(out=slab2[:], in_=w2_e[l, m])
                    ps = ps_m.tile([128, 512], F32, name="pp", tag="psm")
                    for k in range(NF):
                        nc.tensor.matmul(
                            ps[:, 0:T], slab2[:, k, :], r_sb[:, k, :],
                            start=(k == 0), stop=(k == NF - 1))
                    rt = tp.tile([128, T], F32, name="rt2", tag="lntmp")
                    nc.scalar.activation(
                        rt[:], ps[:, 0:T], AF.Identity,
                        bias=par[:, PC_B2 + m:PC_B2 + m + 1])
                    nc.vector.tensor_add(x_sb[:, m, :], x_sb[:, m, :], rt[:])
                    ln_stats(st1, st2, m)

            # =================== final LN + sharded vocab projection ========
            ln_apply(st1, st2, fin_sb, 0, 8, h_sb)
            for tb in range(NT):
                nc.sync.dma_start(
                    out=h_local[tb].rearrange("p (k t) -> p k t", k=ND),
                    in_=h_sb[:, :, 128 * tb:128 * tb + 128])
            nc.gpsimd.collective_compute(
                "AllGather", ALU.bypass,
                replica_groups=[[0, 1, 2, 3], [4, 5, 6, 7]],
                ins=[h_local[:].opt()], outs=[h_gath[:].opt()])
            # gathered h for all 1024 group tokens, GATHER block order
            nc.scalar.dma_start(
                out=kt_sb[:].rearrange("p g k t -> p g (k t)"),
                in_=h_gath[:].rearrange("g p c -> p g c"))

            for vs in range(NVG):
                slab = wp.tile([128, ND, 512], F16, name="wslab", tag="wslab")
                nc.sync.dma_start(out=slab[:, :, 0:VSL], in_=wout_e[vs])
                bo_t = sp.tile([1, 512], F32, name="bo_t", tag="bo")
                nc.sync.dma_start(out=bo_t[0:1, 0:VSL],
                                  in_=bout_e[0:1, VSL * vs:VSL * vs + VSL])
                bb = ps_u.tile([128, 512], F32, name="bb", tag="psu")
                nc.tensor.matmul(bb[:, 0:VSL], ones_sb[0:1, 0:128],
                                 bo_t[0:1, 0:VSL], start=True, stop=True)
                bbs = op_.tile([128, 512], F32, name="bbs", tag="outt")
                nc.scalar.copy(bbs[:, 0:VSL], bb[:, 0:VSL])
                for tb in range(NB):
                    row = CAN[tb]
                    ps = ps_m.tile([128, 512], F32, name="pp", tag="psm")
                    for k in range(ND):
                        nc.tensor.matmul(
                            ps[:, 0:VSL], kt_sb[:, tb, k, :],
                            slab[:, k, 0:VSL],
                            start=(k == 0), stop=(k == ND - 1))
                    ot = op_.tile([128, 512], F32, name="ot", tag="outt")
                    nc.vector.tensor_add(ot[:, 0:VSL], ps[:, 0:VSL],
                                         bbs[:, 0:VSL])
                    nc.sync.dma_start(
                        out=out_e[128 * row:128 * row + 128,
                                  VSL * vs:VSL * vs + VSL],
                        in_=ot[:, 0:VSL])
    return nc


def _to16(a):
    return np.asarray(a, np.float32).astype(np.float16)


def _cols(v, n):
    Lx = v.shape[0]
    return np.asarray(v, np.float32).reshape(Lx, n, 128).transpose(0, 2, 1)


def _sw(w, nslab, width):
    """[..., D_in, N] row-major -> [..., nslab, 128, D_in//128, width]."""
    lead = w.shape[:-2]
    din, n = w.shape[-2], w.shape[-1]
    k = din // 128
    assert n == nslab * width
    w = w.reshape(*lead, k, 128, nslab, width)
    order = tuple(range(len(lead))) + tuple(
        len(lead) + i for i in (2, 1, 0, 3))
    return np.ascontiguousarray(w.transpose(order))


def prepare_inputs(inputs):
    ids = np.asarray(inputs["input_ids"]).astype(np.int32)
    pos = np.asarray(inputs["pos_emb"], np.float32)[:S]

    par = np.concatenate([
        _cols(inputs["bq"], ND), _cols(inputs["bk"], ND),
        _cols(inputs["bo"], ND), _cols(inputs["b1"], NF),
        _cols(inputs["b2"], ND), _cols(inputs["ln1_g"], ND),
        _cols(inputs["ln1_b"], ND), _cols(inputs["ln2_g"], ND),
        _cols(inputs["ln2_b"], ND)], axis=2).astype(np.float32)
    assert par.shape == (L, 128, NPC)

    fin = np.concatenate([
        np.asarray(inputs["lnf_g"], np.float32).reshape(ND, 128).T,
        np.asarray(inputs["lnf_b"], np.float32).reshape(ND, 128).T],
        axis=1).astype(np.float32)

    wout16 = _to16(inputs["Wout"])
    bout = np.asarray(inputs["bout"], np.float32)

    shared = {
        "tok_emb": np.ascontiguousarray(np.asarray(inputs["tok_emb"],
                                                   np.float32)),
        "Wq": _sw(_to16(inputs["Wq"]), 2, 512),
        "Wk": _sw(_to16(inputs["Wk"]), 2, 512),
        "Wv": _sw(_to16(inputs["Wv"]), 2, 512),
        "Wo": _sw(_to16(inputs["Wo"]), 2, 512),
        "W1": _sw(_to16(inputs["W1"]), 8, 512),
        "W2": _sw(_to16(inputs["W2"]), ND, 128),
        "par": par,
        "bv": np.asarray(inputs["bv"], np.float32).reshape(L, 1, D),
        "fin": fin,
    }

    in_maps = []
    kk = np.arange(128)[:, None]                      # key within block
    qq = np.arange(128)[None, :]                      # query within block
    for c in range(NCORES):
        b, ch = c // G, c % G
        a_blk, b_blk = ch, 7 - ch
        tokA = slice(128 * a_blk, 128 * a_blk + 128)
        tokB = slice(128 * b_blk, 128 * b_blk + 128)
        ids_c = np.ascontiguousarray(
            np.stack([ids[b, tokA], ids[b, tokB]], axis=1))
        pos_c = np.ascontiguousarray(
            np.concatenate([pos[tokA], pos[tokB]], axis=0)
            .T.reshape(ND, 128, T).transpose(1, 0, 2))
        # masks [128 keys, 1536]: blocks 0..3 x [A|B] queries, 4..7 x B
        m = np.zeros((128, EW), np.float32)
        for blk in range(4):
            kg = 128 * blk + kk
            m[:, 256 * blk:256 * blk + 128] = kg <= (128 * a_blk + qq)
            m[:, 256 * blk + 128:256 * blk + 256] = kg <= (128 * b_blk + qq)
        for blk in range(4, 8):
            kg = 128 * blk + kk
            m[:, 1024 + 128 * (blk - 4):1152 + 128 * (blk - 4)] = \
                kg <= (128 * b_blk + qq)
        wout_c = _sw(wout16[:, VS * ch:VS * ch + VS], NVG, VSL)
        in_maps.append({
            "ids": ids_c, "pos_t": pos_c,
            "masks": np.ascontiguousarray(m.astype(ml_dtypes.bfloat16)),
            "Wout": wout_c,
            "bout": np.ascontiguousarray(bout[VS * ch:VS * ch + VS]
                                         .reshape(1, VS)),
            **shared})
    return in_maps


def run(inputs, trace=False):
    if "nc" not in _cache:
        nc = build()
        nc.compile()
        _cache["nc"] = nc
    nc = _cache["nc"]
    in_maps = prepare_inputs(inputs)
    res = run_bass_kernel_spmd(nc, in_maps, core_ids=list(range(NCORES)),
                               trace=trace)
    full = np.empty((B, S, V), np.float32)
    for c in range(NCORES):
        b, ch = c // G, c % G
        full[b, :, VS * ch:VS * ch + VS] = res.results[c]["out"]
    return full, res


def kernel(**inputs):
    full, _ = run(inputs, trace=False)
    return full


# revision 12
# speedup vs baseline: 1.1414x; 1.0020x over previous
"""Distributed 8-layer dense transformer on 8 TRN2 NeuronCores — v2.

Sharding: zigzag context-parallel. Each 4-core group owns one batch
element (1024 tokens = 8 blocks of 128); core c owns blocks (c, 7-c)
("A" and "B" halves, 256 tokens total). This makes causal attention
UNIFORM across cores: A needs key-blocks 0..3, B needs 0..7 (12
block-passes vs 16 for full attention), with per-core masks as data.
All weights replicated; per layer ONE fused K+V AllGather (fp16) per
4-core group. The vocab projection is Megatron-sharded: final h is
AllGathered and each core computes all 1024 group tokens x its own
8000-wide vocab slice (4x less Wout DMA).

Layouts: activations feature-major (x^T: [D, T]). V token-major with
an appended ones-column per head so the softmax denominator rides the
AV matmul. Weights are host-swizzled to [128, k, n] so every slab DMA
is one fat contiguous run per partition.

Attention per head: 8 scores MMs (blocks 0-3 vs all 256 q, blocks 4-7
vs B's 128 q) into 3 PSUM banks; 3 batched exps (Act); mask-mul on the
Pool engine; 8 AV MMs into ONE [65,256] PSUM tile (B-only blocks
accumulate onto cols 128:256); recip+scale on DVE. AV runs one head
behind scores so the PE never waits on Act (keeps the HAM clock gate
released).

PSUM rule: a matmul with start=True clears has_written for its whole
bank, so two interleaved accumulation groups must not share a bank.

Precision: fp16 weights/activations, bf16 exp tiles and V, f32
residual stream / LN stats / PSUM accumulation.
"""

import numpy as np
import ml_dtypes

import concourse.bass as bass
import concourse.mybir as mybir
import concourse.tile as tile
import concourse.bacc as bacc
from concourse.bass_utils import run_bass_kernel_spmd

F32 = mybir.dt.float32
F16 = mybir.dt.float16
BF16 = mybir.dt.bfloat16
I32 = mybir.dt.int32
AF = mybir.ActivationFunctionType
ALU = mybir.AluOpType

L, D, H, DK, F, V, S, B = 8, 1024, 16, 64, 4096, 32000, 1024, 2
NCORES = 8
G = 4
T = 256                 # tokens per core (two 128-blocks: A then B)
NT = T // 128           # 2
ND = D // 128           # 8
NF = F // 128           # 32
NB = 8                  # key blocks of 128 per batch element
VO = DK + 1             # 65
VS = V // G             # 8000 vocab slice per core
NVG = 16                # vocab slabs of 500
VSL = VS // NVG         # 500
EPS = 1e-5
SCALE = 1.0 / np.sqrt(DK)

KVW = ND * 128 + H * VO     # 1024 + 1040 = 2064 fp16 per (block, partition)
EW = 1536                   # e_t / mask width per head

# gather position of canonical block b (AG rank r contributes blocks r, 7-r)
GP = [0, 2, 4, 6, 7, 5, 3, 1]
CAN = [0, 7, 1, 6, 2, 5, 3, 4]   # canonical block held at gather position g

PC_BQ, PC_BK, PC_BO, PC_B1, PC_B2 = 0, 8, 16, 24, 56
PC_G1, PC_BE1, PC_G2, PC_BE2 = 64, 72, 80, 88
NPC = 96

_cache = {}


def build():
    nc = bacc.Bacc("TRN2", target_bir_lowering=False, debug=False,
                   num_devices=NCORES)

    ids_e = nc.dram_tensor("ids", [128, NT], I32, kind="ExternalInput")
    tok_e = nc.dram_tensor("tok_emb", [V, D], F32, kind="ExternalInput")
    pos_e = nc.dram_tensor("pos_t", [128, ND, T], F32, kind="ExternalInput")
    mask_e = nc.dram_tensor("masks", [128, EW], BF16, kind="ExternalInput")
    wq_e = nc.dram_tensor("Wq", [L, 2, 128, ND, 512], F16, kind="ExternalInput")
    wk_e = nc.dram_tensor("Wk", [L, 2, 128, ND, 512], F16, kind="ExternalInput")
    wv_e = nc.dram_tensor("Wv", [L, 2, 128, ND, 512], F16, kind="ExternalInput")
    wo_e = nc.dram_tensor("Wo", [L, 2, 128, ND, 512], F16, kind="ExternalInput")
    w1_e = nc.dram_tensor("W1", [L, 8, 128, ND, 512], F16, kind="ExternalInput")
    w2_e = nc.dram_tensor("W2", [L, ND, 128, NF, 128], F16, kind="ExternalInput")
    wout_e = nc.dram_tensor("Wout", [NVG, 128, ND, VSL], F16,
                            kind="ExternalInput")
    par_e = nc.dram_tensor("par", [L, 128, NPC], F32, kind="ExternalInput")
    bv_e = nc.dram_tensor("bv", [L, 1, D], F32, kind="ExternalInput")
    fin_e = nc.dram_tensor("fin", [128, 16], F32, kind="ExternalInput")
    bout_e = nc.dram_tensor("bout", [1, VS], F32, kind="ExternalInput")
    out_e = nc.dram_tensor("out", [NB * 128, VS], F32, kind="ExternalOutput")

    ident_c = nc.inline_tensor(np.eye(128, dtype=np.float32), name="identc")
    ones_c = nc.inline_tensor(np.ones((128, 128), dtype=np.float32), name="onesc")

    with tile.TileContext(nc) as tc:
        with (
            tc.tile_pool(name="persist", bufs=1) as pp,
            tc.tile_pool(name="wp", bufs=3) as wp,
            tc.tile_pool(name="w2p", bufs=2) as w2p,
            tc.tile_pool(name="ep", bufs=8) as ep,
            tc.tile_pool(name="small", bufs=3) as sp,
            tc.tile_pool(name="tmpp", bufs=4) as tp,
            tc.tile_pool(name="bcsp", bufs=2) as bcp,
            tc.tile_pool(name="outp", bufs=3) as op_,
            tc.tile_pool(name="embp", bufs=1) as embp,
            tc.tile_pool(name="ps_m", bufs=4, space="PSUM") as ps_m,
            tc.tile_pool(name="ps_o", bufs=2, space="PSUM") as ps_o,
            tc.tile_pool(name="ps_u", bufs=2, space="PSUM") as ps_u,
            tc.tile_pool(name="dram", bufs=1, space="DRAM") as dp,
        ):
            x_sb = pp.tile([128, ND, T], F32, name="x_sb")
            h_sb = pp.tile([128, ND, T], F16, name="h_sb")
            q_sb = pp.tile([128, ND, T], F16, name="q_sb")
            o_sb = pp.tile([128, ND, T], F16, name="o_sb")
            ktl_sb = pp.tile([128, ND, T], F16, name="ktl_sb")
            vl_sb = pp.tile([128, NT, H * VO], BF16, name="vl_sb")
            kt_sb = pp.tile([128, NB, ND, 128], F16, name="kt_sb")
            v_sb = pp.tile([128, NB, H * VO], BF16, name="v_sb")
            r_sb = pp.tile([128, NF, T], F16, name="r_sb")
            mask_sb = pp.tile([128, EW], BF16, name="mask_sb")
            ids_sb = pp.tile([128, NT], I32, name="ids_sb")
            id_sb = pp.tile([128, 128], F32, name="id_sb")
            ones_sb = pp.tile([128, 128], F32, name="ones_sb")
            fin_sb = pp.tile([128, 16], F32, name="fin_sb")
            bvbc_sb = pp.tile([128, D], F32, name="bvbc_sb")
            eps_sb = pp.tile([1, 1], F32, name="eps_sb")

            k_local = dp.tile([NT, 128, ND * 128], F16, name="k_local")
            k_gath = dp.tile([NB, 128, ND * 128], F16, name="k_gath")
            v_local = dp.tile([NT, 128, H * VO], F16, name="v_local")
            v_gath = dp.tile([NB, 128, H * VO], F16, name="v_gath")
            h_local = dp.tile([NT, 128, ND * 128], F16, name="h_local")
            h_gath = dp.tile([NB, 128, ND * 128], F16, name="h_gath")

            nc.sync.dma_start(out=ids_sb[:], in_=ids_e[:])
            nc.sync.dma_start(out=id_sb[:], in_=ident_c[:])
            nc.sync.dma_start(out=ones_sb[:], in_=ones_c[:])
            nc.sync.dma_start(out=mask_sb[:], in_=mask_e[:])
            nc.sync.dma_start(out=fin_sb[:], in_=fin_e[:])
            nc.vector.memset(vl_sb[:], 1.0)
            nc.vector.memset(eps_sb[:], EPS)

            # ---- embedding: gather + transpose to feature-major + pos add
            for tb in range(NT):
                pos_sb = embp.tile([128, ND, 128], F32, name="pos_sb", tag="pos")
                nc.sync.dma_start(out=pos_sb[:], in_=pos_e[:, :, 128 * tb:128 * tb + 128])
                emb = embp.tile([128, D], F32, name="emb")
                nc.gpsimd.indirect_dma_start(
                    out=emb[:], out_offset=None, in_=tok_e[:],
                    in_offset=bass.IndirectOffsetOnAxis(
                        ap=ids_sb[:, tb:tb + 1], axis=0))
                for dt in range(ND):
                    tps = ps_u.tile([128, 512], F32, name="tps", tag="psu")
                    nc.tensor.transpose(
                        tps[:, 0:128], emb[:, 128 * dt:128 * dt + 128], id_sb[:])
                    nc.vector.tensor_add(
                        x_sb[:, dt, 128 * tb:128 * tb + 128],
                        tps[:, 0:128],
                        pos_sb[:, dt, :])

            def ln_stats(st1, st2, k):
                """Accumulate sum (st1) and sumsq (st2) of x_sb[:, k, :].
                Separate PSUM banks (start=True clears bank-wide)."""
                nc.tensor.matmul(st1[0:1, 0:T], ones_sb[:, 0:1],
                                 x_sb[:, k, :], start=(k == 0),
                                 stop=(k == ND - 1))
                sq = tp.tile([128, T], F32, name="sq", tag="lntmp")
                nc.scalar.activation(sq[:], x_sb[:, k, :], AF.Square)
                nc.tensor.matmul(st2[0:1, 0:T], ones_sb[:, 0:1],
                                 sq[:], start=(k == 0), stop=(k == ND - 1))

            def ln_apply(st1, st2, par_ap, gcol, bcol, out_sb):
                """Finish LN from stats: x_sb (f32) -> out_sb (f16)."""
                mr = sp.tile([1, 512], F32, name="mr", tag="mr")
                t1 = sp.tile([1, T], F32, name="lns1", tag="lns")
                t2 = sp.tile([1, T], F32, name="lns2", tag="lns")
                nc.scalar.activation(mr[0:1, 0:T], st1[0:1, 0:T], AF.Copy,
                                     scale=1.0 / D)
                nc.scalar.activation(t1[0:1, :], st2[0:1, 0:T], AF.Copy,
                                     scale=1.0 / D)
                nc.vector.tensor_mul(t2[0:1, :], mr[0:1, 0:T], mr[0:1, 0:T])
                nc.vector.tensor_sub(t1[0:1, :], t1[0:1, :], t2[0:1, :])
                nc.scalar.activation(t2[0:1, :], t1[0:1, :], AF.Sqrt,
                                     bias=eps_sb[0:1, 0:1])
                nc.vector.reciprocal(mr[0:1, T:2 * T], t2[0:1, :])
                bc = ps_u.tile([128, 512], F32, name="lnbc", tag="psu")
                nc.tensor.matmul(bc[:, 0:512], ones_sb[0:1, 0:128],
                                 mr[0:1, 0:512], start=True, stop=True)
                bcs = bcp.tile([128, 512], F32, name="bcs", tag="bcs")
                nc.scalar.copy(bcs[:], bc[:])
                for k in range(ND):
                    u1 = tp.tile([128, T], F32, name="u1", tag="lntmp")
                    u2 = tp.tile([128, T], F32, name="u2", tag="lntmp")
                    eng = nc.vector if k % 2 == 0 else nc.gpsimd
                    eng.tensor_sub(u1[:], x_sb[:, k, :], bcs[:, 0:T])
                    eng.tensor_mul(u2[:], u1[:], bcs[:, T:2 * T])
                    eng.tensor_scalar(
                        out=out_sb[:, k, :], in0=u2[:],
                        scalar1=par_ap[:, gcol + k:gcol + k + 1],
                        scalar2=par_ap[:, bcol + k:bcol + k + 1],
                        op0=ALU.mult, op1=ALU.add)

            def std_proj(w_ext, l, dst_sb, bias_par, bias_col):
                """dst[:, m, :] (f16) = (h^T W)[:, m] + bias, feature-major."""
                for c in range(2):
                    slab = wp.tile([128, ND, 512], F16, name="wslab", tag="wslab")
                    nc.sync.dma_start(out=slab[:], in_=w_ext[l, c])
                    for mm in range(4):
                        m = 4 * c + mm
                        ps = ps_m.tile([128, 512], F32, name="pp", tag="psm")
                        for k in range(ND):
                            nc.tensor.matmul(
                                ps[:, 0:T],
                                slab[:, k, 128 * mm:128 * mm + 128],
                                h_sb[:, k, :],
                                start=(k == 0), stop=(k == ND - 1))
                        nc.scalar.activation(
                            dst_sb[:, m, :], ps[:, 0:T], AF.Identity,
                            bias=bias_par[:, bias_col + m:bias_col + m + 1])

            # LN1 stats for layer 0 come from the embedding output
            st1 = ps_u.tile([1, 512], F32, name="st1", tag="psu")
            st2 = ps_u.tile([1, 512], F32, name="st2", tag="psu")
            for k in range(ND):
                ln_stats(st1, st2, k)

            # =================== layers ===================
            for l in range(L):
                par = sp.tile([128, NPC], F32, name="par", tag="par")
                nc.sync.dma_start(out=par[:], in_=par_e[l])
                bv_t = sp.tile([1, D], F32, name="bv_t", tag="bv")
                nc.sync.dma_start(out=bv_t[:], in_=bv_e[l])

                # ---- LN1 (stats were accumulated by the previous loop)
                ln_apply(st1, st2, par, PC_G1, PC_BE1, h_sb)

                # ---- K projection, then V projection, then ONE fused AG
                std_proj(wk_e, l, ktl_sb, par, PC_BK)

                # bv broadcast here: by now ln_apply's bc consumers are done,
                # so the ps_u rotation cannot deadlock across engines
                for c in range(2):
                    bcv = ps_u.tile([128, 512], F32, name="bcv", tag="psu")
                    nc.tensor.matmul(bcv[:], ones_sb[0:1, 0:128],
                                     bv_t[0:1, 512 * c:512 * c + 512],
                                     start=True, stop=True)
                    nc.scalar.copy(bvbc_sb[:, 512 * c:512 * c + 512], bcv[:])
                for tb in range(NT):
                    nc.sync.dma_start(
                        out=k_local[tb].rearrange("p (k t) -> p k t", k=ND),
                        in_=ktl_sb[:, :, 128 * tb:128 * tb + 128])
                nc.gpsimd.collective_compute(
                    "AllGather", ALU.bypass,
                    replica_groups=[[0, 1, 2, 3], [4, 5, 6, 7]],
                    ins=[k_local[:].opt()], outs=[k_gath[:].opt()])

                # V projection (token-major, reversed)
                for c in range(2):
                    slab = wp.tile([128, ND, 512], F16, name="wslab", tag="wslab")
                    nc.sync.dma_start(out=slab[:], in_=wv_e[l, c])
                    for tb in range(NT):
                        ps = ps_m.tile([128, 512], F32, name="pp", tag="psm")
                        for k in range(ND):
                            nc.tensor.matmul(
                                ps[:], h_sb[:, k, 128 * tb:128 * tb + 128],
                                slab[:, k, :],
                                start=(k == 0), stop=(k == ND - 1))
                        dst = vl_sb[:, tb,
                                    VO * 8 * c:VO * 8 * c + VO * 8].rearrange(
                            "p (j v) -> p j v", v=VO)[:, :, 0:DK]
                        nc.vector.tensor_add(
                            dst,
                            ps[:].rearrange("p (j v) -> p j v", v=DK),
                            bvbc_sb[:, 512 * c:512 * c + 512].rearrange(
                                "p (j v) -> p j v", v=DK))
                for tb in range(NT):
                    nc.sync.dma_start(
                        out=v_local[tb],
                        in_=vl_sb[:, tb, :].bitcast(F16))
                nc.gpsimd.collective_compute(
                    "AllGather", ALU.bypass,
                    replica_groups=[[0, 1, 2, 3], [4, 5, 6, 7]],
                    ins=[v_local[:].opt()], outs=[v_gath[:].opt()])

                # ---- Q projection (overlaps the AllGather)
                std_proj(wq_e, l, q_sb, par, PC_BQ)

                # ---- HAM-warming filler: keep the PE clock gate released
                #      through the AG-K wait; results never read.
                warm = ps_o.tile([VO, T], F32, name="warm", tag="pso")
                for _ in range(88):
                    nc.tensor.matmul(warm[0:64, 0:T], h_sb[:, 0, 0:64],
                                     h_sb[:, 0, :], start=True, stop=True)

                # ---- pull gathered K^T / V into SBUF: ONE dma each, on the
                #      Act engine's queue (sync keeps prefetching weights).
                #      SBUF holds blocks in GATHER order; loops index via GP.
                nc.scalar.dma_start(
                    out=kt_sb[:].rearrange("p g k t -> p g (k t)"),
                    in_=k_gath[:].rearrange("g p c -> p g c"))
                nc.scalar.dma_start(
                    out=v_sb[:].bitcast(F16),
                    in_=v_gath[:].rearrange("g p c -> p g c"))

                # ---- attention: 12 block-passes per head, AV one head behind
                def attn_scores(h):
                    po, pt = 64 * (h % 2), h // 2
                    e_t = ep.tile([128, EW], BF16, name="e_t", tag="et")
                    # blocks 0..3 vs all 256 queries -> 2 banks
                    for half in range(2):
                        sa = ps_m.tile([128, 512], F32, name="sa", tag="psm")
                        for i in range(2):
                            b = 2 * half + i
                            nc.tensor.matmul(
                                sa[:, 256 * i:256 * i + 256],
                                kt_sb[po:po + 64, GP[b], pt, :],
                                q_sb[po:po + 64, pt, :],
                                start=True, stop=True)
                        nc.scalar.activation(
                            e_t[:, 512 * half:512 * half + 512], sa[:], AF.Exp,
                            scale=float(SCALE))
                        nc.vector.tensor_mul(
                            e_t[:, 512 * half:512 * half + 512],
                            e_t[:, 512 * half:512 * half + 512],
                            mask_sb[:, 512 * half:512 * half + 512])
                    # blocks 4..7 vs B queries only -> 1 bank
                    sa = ps_m.tile([128, 512], F32, name="sa", tag="psm")
                    for i in range(4):
                        b = 4 + i
                        nc.tensor.matmul(
                            sa[:, 128 * i:128 * i + 128],
                            kt_sb[po:po + 64, GP[b], pt, :],
                            q_sb[po:po + 64, pt, 128:256],
                            start=True, stop=True)
                    nc.scalar.activation(
                        e_t[:, 1024:1536], sa[:], AF.Exp, scale=float(SCALE))
                    nc.gpsimd.tensor_mul(e_t[:, 1024:1536], e_t[:, 1024:1536],
                                         mask_sb[:, 1024:1536])
                    return e_t

                def attn_av(h, e_t):
                    po, pt = 64 * (h % 2), h // 2
                    oo = ps_o.tile([VO, T], F32, name="oo", tag="pso")
                    for b in range(4):
                        nc.tensor.matmul(
                            oo[:], v_sb[:, GP[b], VO * h:VO * h + VO],
                            e_t[:, 256 * b:256 * b + 256],
                            start=(b == 0), stop=False)
                    for b in range(4):
                        nc.tensor.matmul(
                            oo[:, 128:256], v_sb[:, GP[4 + b], VO * h:VO * h + VO],
                            e_t[:, 1024 + 128 * b:1152 + 128 * b],
                            start=False, stop=(b == 3))
                    rec = sp.tile([1, T], F32, name="rec", tag="rec")
                    nc.vector.reciprocal(rec[0:1, :], oo[DK:VO, :])
                    rbc = ps_u.tile([128, 512], F32, name="rbc", tag="psu")
                    nc.tensor.matmul(rbc[0:64, 0:T], ones_sb[0:1, 0:64],
                                     rec[0:1, :], start=True, stop=True)
                    rbs = tp.tile([64, T], F32, name="rbs", tag="rbs")
                    nc.vector.tensor_copy(out=rbs[:], in_=rbc[0:64, 0:T])
                    nc.vector.tensor_mul(o_sb[po:po + 64, pt, :],
                                         oo[0:DK, :], rbs[:])

                pend = []
                for h in range(H):
                    e_t = attn_scores(h)
                    pend.append((h, e_t))
                    if len(pend) > 2:
                        attn_av(*pend.pop(0))
                for pe_ in pend:
                    attn_av(*pe_)

                # ---- attention output projection + residual + LN2 stats
                st1 = ps_u.tile([1, 512], F32, name="st1", tag="psu")
                st2 = ps_u.tile([1, 512], F32, name="st2", tag="psu")
                for c in range(2):
                    slab = wp.tile([128, ND, 512], F16, name="wslab", tag="wslab")
                    nc.sync.dma_start(out=slab[:], in_=wo_e[l, c])
                    for mm in range(4):
                        m = 4 * c + mm
                        ps = ps_m.tile([128, 512], F32, name="pp", tag="psm")
                        for k in range(ND):
                            nc.tensor.matmul(
                                ps[:, 0:T],
                                slab[:, k, 128 * mm:128 * mm + 128],
                                o_sb[:, k, :],
                                start=(k == 0), stop=(k == ND - 1))
                        rt = tp.tile([128, T], F32, name="rt", tag="lntmp")
                        nc.scalar.activation(
                            rt[:], ps[:, 0:T], AF.Identity,
                            bias=par[:, PC_BO + m:PC_BO + m + 1])
                        nc.vector.tensor_add(x_sb[:, m, :], x_sb[:, m, :], rt[:])
                        ln_stats(st1, st2, m)

                # ---- LN2
                ln_apply(st1, st2, par, PC_G2, PC_BE2, h_sb)

                # ---- FFN W1 + relu
                for c in range(8):
                    slab = wp.tile([128, ND, 512], F16, name="wslab", tag="wslab")
                    nc.sync.dma_start(out=slab[:], in_=w1_e[l, c])
                    for mm in range(4):
                        ot = 4 * c + mm
                        ps = ps_m.tile([128, 512], F32, name="pp", tag="psm")
                        for k in range(ND):
                            nc.tensor.matmul(
                                ps[:, 0:T],
                                slab[:, k, 128 * mm:128 * mm + 128],
                                h_sb[:, k, :],
                                start=(k == 0), stop=(k == ND - 1))
                        nc.scalar.activation(
                            r_sb[:, ot, :], ps[:, 0:T], AF.Relu,
                            bias=par[:, PC_B1 + ot:PC_B1 + ot + 1])

                # ---- FFN W2 + residual + next-LN stats
                st1 = ps_u.tile([1, 512], F32, name="st1", tag="psu")
                st2 = ps_u.tile([1, 512], F32, name="st2", tag="psu")
                for m in range(ND):
                    slab2 = w2p.tile([128, NF, 128], F16, name="w2slab",
                                     tag="w2slab")
                    nc.sync.dma_start(out=slab2[:], in_=w2_e[l, m])
                    ps = ps_m.tile([128, 512], F32, name="pp", tag="psm")
                    for k in range(NF):
                        nc.tensor.matmul(
                            ps[:, 0:T], slab2[:, k, :], r_sb[:, k, :],
                            start=(k == 0), stop=(k == NF - 1))
                    rt = tp.tile([128, T], F32, name="rt2", tag="lntmp")
                    nc.scalar.activation(
                        rt[:], ps[:, 0:T], AF.Identity,
                        bias=par[:, PC_B2 + m:PC_B2 + m + 1])
                    nc.vector.tensor_add(x_sb[:, m, :], x_sb[:, m, :], rt[:])
                    ln_stats(st1, st2, m)

            # =================== final LN + sharded vocab projection ========
            ln_apply(st1, st2, fin_sb, 0, 8, h_sb)
            for tb in range(NT):
                nc.sync.dma_start(
                    out=h_local[tb].rearrange("p (k t) -> p k t", k=ND),
                    in_=h_sb[:, :, 128 * tb:128 * tb + 128])
            nc.gpsimd.collective_compute(
                "AllGather", ALU.bypass,
                replica_groups=[[0, 1, 2, 3], [4, 5, 6, 7]],
                ins=[h_local[:].opt()], outs=[h_gath[:].opt()])
            # gathered h for all 1024 group tokens, GATHER block order
            nc.scalar.dma_start(
                out=kt_sb[:].rearrange("p g k t -> p g (k t)"),
                in_=h_gath[:].rearrange("g p c -> p g c"))

            for vs in range(NVG):
                slab = wp.tile([128, ND, 512], F16, name="wslab", tag="wslab")
                nc.sync.dma_start(out=slab[:, :, 0:VSL], in_=wout_e[vs])
                bo_t = sp.tile([1, 512], F32, name="bo_t", tag="bo")
                nc.sync.dma_start(out=bo_t[0:1, 0:VSL],
                                  in_=bout_e[0:1, VSL * vs:VSL * vs + VSL])
                bb = ps_u.tile([128, 512], F32, name="bb", tag="psu")
                nc.tensor.matmul(bb[:, 0:VSL], ones_sb[0:1, 0:128],
                                 bo_t[0:1, 0:VSL], start=True, stop=True)
                bbs = op_.tile([128, 512], F32, name="bbs", tag="outt")
                nc.scalar.copy(bbs[:, 0:VSL], bb[:, 0:VSL])
                for tb in range(NB):
                    row = CAN[tb]
                    ps = ps_m.tile([128, 512], F32, name="pp", tag="psm")
                    for k in range(ND):
                        nc.tensor.matmul(
                            ps[:, 0:VSL], kt_sb[:, tb, k, :],
                            slab[:, k, 0:VSL],
                            start=(k == 0), stop=(k == ND - 1))
                    ot = op_.tile([128, 512], F32, name="ot", tag="outt")
                    nc.vector.tensor_add(ot[:, 0:VSL], ps[:, 0:VSL],
                                         bbs[:, 0:VSL])
                    nc.sync.dma_start(
                        out=out_e[128 * row:128 * row + 128,
                                  VSL * vs:VSL * vs + VSL],
                        in_=ot[:, 0:VSL])
    return nc


def _to16(a):
    return np.asarray(a, np.float32).astype(np.float16)


def _cols(v, n):
    Lx = v.shape[0]
    return np.asarray(v, np.float32).reshape(Lx, n, 128).transpose(0, 2, 1)


def _sw(w, nslab, width):
    """[..., D_in, N] row-major -> [..., nslab, 128, D_in//128, width]."""
    lead = w.shape[:-2]
    din, n = w.shape[-2], w.shape[-1]
    k = din // 128
    assert n == nslab * width
    w = w.reshape(*lead, k, 128, nslab, width)
    order = tuple(range(len(lead))) + tuple(
        len(lead) + i for i in (2, 1, 0, 3))
    return np.ascontiguousarray(w.transpose(order))


def prepare_inputs(inputs):
    ids = np.asarray(inputs["input_ids"]).astype(np.int32)
    pos = np.asarray(inputs["pos_emb"], np.float32)[:S]

    par = np.concatenate([
        _cols(inputs["bq"], ND), _cols(inputs["bk"], ND),
        _cols(inputs["bo"], ND), _cols(inputs["b1"], NF),
        _cols(inputs["b2"], ND), _cols(inputs["ln1_g"], ND),
        _cols(inputs["ln1_b"], ND), _cols(inputs["ln2_g"], ND),
        _cols(inputs["ln2_b"], ND)], axis=2).astype(np.float32)
    assert par.shape == (L, 128, NPC)

    fin = np.concatenate([
        np.asarray(inputs["lnf_g"], np.float32).reshape(ND, 128).T,
        np.asarray(inputs["lnf_b"], np.float32).reshape(ND, 128).T],
        axis=1).astype(np.float32)

    wout16 = _to16(inputs["Wout"])
    bout = np.asarray(inputs["bout"], np.float32)

    shared = {
        "tok_emb": np.ascontiguousarray(np.asarray(inputs["tok_emb"],
                                                   np.float32)),
        "Wq": _sw(_to16(inputs["Wq"]), 2, 512),
        "Wk": _sw(_to16(inputs["Wk"]), 2, 512),
        "Wv": _sw(_to16(inputs["Wv"]), 2, 512),
        "Wo": _sw(_to16(inputs["Wo"]), 2, 512),
        "W1": _sw(_to16(inputs["W1"]), 8, 512),
        "W2": _sw(_to16(inputs["W2"]), ND, 128),
        "par": par,
        "bv": np.asarray(inputs["bv"], np.float32).reshape(L, 1, D),
        "fin": fin,
    }

    in_maps = []
    kk = np.arange(128)[:, None]                      # key within block
    qq = np.arange(128)[None, :]                      # query within block
    for c in range(NCORES):
        b, ch = c // G, c % G
        a_blk, b_blk = ch, 7 - ch
        tokA = slice(128 * a_blk, 128 * a_blk + 128)
        tokB = slice(128 * b_blk, 128 * b_blk + 128)
        ids_c = np.ascontiguousarray(
            np.stack([ids[b, tokA], ids[b, tokB]], axis=1))
        pos_c = np.ascontiguousarray(
            np.concatenate([pos[tokA], pos[tokB]], axis=0)
            .T.reshape(ND, 128, T).transpose(1, 0, 2))
        # masks [128 keys, 1536]: blocks 0..3 x [A|B] queries, 4..7 x B
        m = np.zeros((128, EW), np.float32)
        for blk in range(4):
            kg = 128 * blk + kk
            m[:, 256 * blk:256 * blk + 128] = kg <= (128 * a_blk + qq)
            m[:, 256 * blk + 128:256 * blk + 256] = kg <= (128 * b_blk + qq)
        for blk in range(4, 8):
            kg = 128 * blk + kk
            m[:, 1024 + 128 * (blk - 4):1152 + 128 * (blk - 4)] = \
                kg <= (128 * b_blk + qq)
        wout_c = _sw(wout16[:, VS * ch:VS * ch + VS], NVG, VSL)
        in_maps.append({
            "ids": ids_c, "pos_t": pos_c,
            "masks": np.ascontiguousarray(m.astype(ml_dtypes.bfloat16)),
            "Wout": wout_c,
            "bout": np.ascontiguousarray(bout[VS * ch:VS * ch + VS]
                                         .reshape(1, VS)),
            **shared})
    return in_maps


def run(inputs, trace=False):
    if "nc" not in _cache:
        nc = build()
        nc.compile()
        _cache["nc"] = nc
    nc = _cache["nc"]
    in_maps = prepare_inputs(inputs)
    res = run_bass_kernel_spmd(nc, in_maps, core_ids=list(range(NCORES)),
                               trace=trace)
    full = np.empty((B, S, V), np.float32)
    for c in range(NCORES):
        b, ch = c // G, c % G
        full[b, :, VS * ch:VS * ch + VS] = res.results[c]["out"]
    return full, res


def kernel(**inputs):
    full, _ = run(inputs, trace=False)
    return full
